# revision 1
# baseline (speedup 1.0000x reference)
"""GAT (2-layer) + mean-pool + linear head on 8 Trainium2 NeuronCores.

Strategy (data-parallel over graphs, per the sharding hint):
  - Nodes/graphs are split into 8 contiguous ranges (batch is sorted), one per
    core; each core owns its graphs' dst-nodes and the edges targeting them.
  - 3 SPMD launches:
      A: per-node  [W1|a_s1|a_d1]^T @ x^T              -> h1, as1, ad1
      B: L1 edge aggregation (segment softmax via one-hot scatter-matmuls,
         PSUM-accumulated per 128-dst tile) + L2 node compute -> h2, as2, ad2
      C: L2 edge aggregation + graph mean-pool (matmul with 0/1 membership
         weights) + linear head -> logits
  - Host glue between launches does the static-index shard/expand work
    (edge->slot layout, per-edge src/dst expansions) so the device consumes
    only dense sequential streams; all arithmetic runs on device.
"""

import sys

sys.path.insert(0, "/opt/trn_rl_repo")

import numpy as np
import ml_dtypes

import concourse.bacc as bacc
import concourse.mybir as mybir
import concourse.tile as tile
from concourse import bass_utils

F32 = mybir.dt.float32
BF16 = mybir.dt.bfloat16

N = 50000
E = 800000
F_IN, F_HID, F_OUT, N_CLS = 128, 64, 64, 10
N_GRAPHS = 512
NEG_SLOPE = 0.2
EPS = 1e-16
N_CORES = 8
P = 128
G_SLOTS = 128

_cache = {}
LAST_LAUNCH_WALLS = []


def _run(nc, in_maps, cores):
    import time
    t0 = time.time()
    res = bass_utils.run_bass_kernel_spmd(nc, in_maps, core_ids=cores)
    LAST_LAUNCH_WALLS.append(time.time() - t0)
    return res


# ----------------------------------------------------------------- launch A
def build_A(nodes_pad):
    nc = bacc.Bacc("TRN2", target_bir_lowering=False, debug=False,
                   num_devices=N_CORES)
    xT = nc.dram_tensor("xT", [P, nodes_pad], F32, kind="ExternalInput").ap()
    w1 = nc.dram_tensor("w1aug", [P, F_HID + 2], F32, kind="ExternalInput").ap()
    out = nc.dram_tensor("node1", [F_HID + 2, nodes_pad], F32,
                         kind="ExternalOutput").ap()
    CH = 512
    with tile.TileContext(nc) as tc:
        with (
            tc.tile_pool(name="sb", bufs=2) as sb,
            tc.tile_pool(name="ps", bufs=2, space="PSUM") as ps,
            tc.tile_pool(name="w", bufs=1) as wp,
        ):
            wt = wp.tile([P, F_HID + 2], F32)
            nc.sync.dma_start(wt[:], w1[:, :])
            ot = wp.tile([F_HID + 2, nodes_pad], F32)
            for c0 in range(0, nodes_pad, CH):
                c1 = min(c0 + CH, nodes_pad)
                xt = sb.tile([P, CH], F32, tag="x")
                nc.sync.dma_start(xt[:, : c1 - c0], xT[:, c0:c1])
                pt = ps.tile([F_HID + 2, CH], F32, tag="p")
                nc.tensor.matmul(pt[:, : c1 - c0], lhsT=wt[:],
                                 rhs=xt[:, : c1 - c0], start=True, stop=True)
                nc.vector.tensor_copy(ot[:, c0:c1], pt[:, : c1 - c0])
            nc.sync.dma_start(out[:, :], ot[:])
    nc.compile()
    return nc


# ------------------------------------------------------------- edge launches
def build_edge(n_tiles, b_uni, is_final, nodes_pad):
    """B (is_final=False): L1 aggregation + L2 node compute.
       C (is_final=True):  L2 aggregation + pooling + head."""
    nc = bacc.Bacc("TRN2", target_bir_lowering=False, debug=False,
                   num_devices=N_CORES)
    TB = int(np.sum(b_uni))
    cpre = np.concatenate([[0], np.cumsum(b_uni)]).astype(int)

    REC = F_HID + 1  # [1 | h] per edge: ones column folds the softmax
    he = nc.dram_tensor("h_edges", [P, TB * REC], BF16,
                        kind="ExternalInput").ap()
    zs = nc.dram_tensor("z", [P, TB], F32, kind="ExternalInput").ap()
    dl = nc.dram_tensor("dst_local", [P, TB], F32, kind="ExternalInput").ap()
    iota_in = nc.dram_tensor("iota", [P, P], BF16, kind="ExternalInput").ap()
    if not is_final:
        brep = nc.dram_tensor("b_rep", [P, F_HID], F32,
                              kind="ExternalInput").ap()
        waug = nc.dram_tensor("w2aug", [F_HID, F_OUT + 2], F32,
                              kind="ExternalInput").ap()
        out = nc.dram_tensor("node2", [F_OUT + 2, nodes_pad], F32,
                             kind="ExternalOutput").ap()
    else:
        brep = nc.dram_tensor("b_rep", [P, F_OUT], F32,
                              kind="ExternalInput").ap()
        poolw = nc.dram_tensor("poolw", [P, n_tiles * G_SLOTS], F32,
                               kind="ExternalInput").ap()
        rcnt = nc.dram_tensor("rcnt", [G_SLOTS, 1], F32,
                              kind="ExternalInput").ap()
        wlin = nc.dram_tensor("wlin", [F_OUT, N_CLS], F32,
                              kind="ExternalInput").ap()
        blin = nc.dram_tensor("blin", [N_CLS, 1], F32,
                              kind="ExternalInput").ap()
        out = nc.dram_tensor("logits", [N_CLS, G_SLOTS], F32,
                             kind="ExternalOutput").ap()

    NSEG = 8
    seg_blocks = (TB + NSEG - 1) // NSEG

    with tile.TileContext(nc) as tc:
        with (
            tc.tile_pool(name="big", bufs=1) as big,
            tc.tile_pool(name="sb", bufs=3) as sb,
            tc.tile_pool(name="oh", bufs=6) as ohp,
            tc.tile_pool(name="accn", bufs=2, space="PSUM") as accnp,
            tc.tile_pool(name="pst", bufs=1, space="PSUM") as pst,
            tc.tile_pool(name="psn", bufs=1, space="PSUM") as psn,
            tc.tile_pool(name="pp", bufs=1, space="PSUM") as ppool,
        ):
            # persistent inputs
            iota_t = big.tile([P, P], BF16)
            nc.sync.dma_start(iota_t[:], iota_in[:, :])
            z_t = big.tile([P, TB], F32)
            nc.sync.dma_start(z_t[:], zs[:, :])
            dl_t = big.tile([P, TB], F32)
            nc.sync.dma_start(dl_t[:], dl[:, :])
            br_t = big.tile([P, brep.shape[1]], F32)
            nc.sync.dma_start(br_t[:], brep[:, :])
            ident = big.tile([P, P], F32)
            from concourse.masks import make_identity
            make_identity(nc, ident[:])
            if not is_final:
                wa_t = big.tile([F_HID, F_OUT + 2], F32)
                nc.sync.dma_start(wa_t[:], waug[:, :])
                n2_t = big.tile([F_OUT + 2, nodes_pad], F32)
            else:
                pw_t = big.tile([P, n_tiles * G_SLOTS], F32)
                nc.sync.dma_start(pw_t[:], poolw[:, :])
                rc_t = big.tile([G_SLOTS, 1], F32)
                nc.sync.dma_start(rc_t[:], rcnt[:, :])
                wl_t = big.tile([F_OUT, N_CLS], F32)
                nc.sync.dma_start(wl_t[:], wlin[:, :])
                bl_t = big.tile([N_CLS, 1], F32)
                nc.sync.dma_start(bl_t[:], blin[:, :])
                pool_ps = ppool.tile([G_SLOTS, F_OUT], F32)

            # e_l = exp(leaky_relu(z)) for the whole stream
            el_t = big.tile([P, TB], F32)
            tmp_t = big.tile([P, TB], F32)
            nc.vector.tensor_scalar_mul(tmp_t[:], z_t[:], NEG_SLOPE)
            nc.vector.tensor_tensor(out=tmp_t[:], in0=tmp_t[:], in1=z_t[:],
                                    op=mybir.AluOpType.max)
            nc.scalar.activation(el_t[:], tmp_t[:],
                                 mybir.ActivationFunctionType.Exp)

            # segmented load of the gathered h stream
            segs = []
            for s in range(NSEG):
                b0, b1 = s * seg_blocks, min((s + 1) * seg_blocks, TB)
                st = big.tile([P, (b1 - b0) * REC], BF16, tag=f"seg{s}")
                nc.sync.dma_start(st[:], he[:, b0 * REC:b1 * REC])
                segs.append((b0, st))

            for t in range(n_tiles):
                accn = accnp.tile([P, REC], F32, tag="accn")
                nb = int(b_uni[t])
                for b in range(nb):
                    c = int(cpre[t]) + b
                    oh = ohp.tile([P, P], BF16, tag="oh")
                    nc.vector.tensor_scalar(
                        oh[:], iota_t[:], dl_t[:, c:c + 1], el_t[:, c:c + 1],
                        mybir.AluOpType.is_equal, mybir.AluOpType.mult)
                    s = c // seg_blocks
                    b0, st = segs[s]
                    rhs = st[:, (c - b0) * REC:(c - b0 + 1) * REC]
                    nc.tensor.matmul(accn[:], lhsT=oh[:], rhs=rhs,
                                     start=(b == 0), stop=(b == nb - 1))
                # epilogue for this dst tile
                den = sb.tile([P, 1], F32, tag="den")
                nc.vector.tensor_scalar_add(den[:], accn[:, 0:1], EPS)
                rec = sb.tile([P, 1], F32, tag="rec")
                nc.vector.reciprocal(rec[:], den[:])
                o1 = sb.tile([P, F_HID], F32, tag="o1")
                nc.vector.tensor_scalar_mul(o1[:], accn[:, 1:], rec[:, :1])
                nc.vector.tensor_tensor(out=o1[:], in0=o1[:], in1=br_t[:],
                                        op=mybir.AluOpType.add)
                if not is_final:
                    nc.scalar.activation(o1[:], o1[:],
                                         mybir.ActivationFunctionType.Relu)
                    tp = pst.tile([F_HID, P], F32, tag="tp")
                    nc.tensor.transpose(tp[:], o1[:], ident[:])
                    hT = sb.tile([F_HID, P], F32, tag="hT")
                    nc.scalar.copy(hT[:], tp[:])
                    pn = psn.tile([F_OUT + 2, P], F32, tag="pn")
                    nc.tensor.matmul(pn[:], lhsT=wa_t[:], rhs=hT[:],
                                     start=True, stop=True)
                    nc.scalar.copy(n2_t[:, t * P:(t + 1) * P], pn[:])
                else:
                    nc.tensor.matmul(
                        pool_ps[:], lhsT=pw_t[:, t * G_SLOTS:(t + 1) * G_SLOTS],
                        rhs=o1[:], start=(t == 0), stop=(t == n_tiles - 1))

            if not is_final:
                nc.sync.dma_start(out[:, :], n2_t[:])
            else:
                pm = sb.tile([G_SLOTS, F_OUT], F32, tag="pm")
                nc.vector.tensor_scalar_mul(pm[:], pool_ps[:], rc_t[:, :1])
                tp2 = pst.tile([F_OUT, G_SLOTS], F32, tag="tp2")
                nc.tensor.transpose(tp2[:], pm[:], ident[:])
                pmT = sb.tile([F_OUT, G_SLOTS], F32, tag="pmT")
                nc.scalar.copy(pmT[:], tp2[:])
                po = psn.tile([N_CLS, G_SLOTS], F32, tag="po")
                nc.tensor.matmul(po[:], lhsT=wl_t[:], rhs=pmT[:],
                                 start=True, stop=True)
                ot = sb.tile([N_CLS, G_SLOTS], F32, tag="ot")
                nc.vector.tensor_scalar_add(ot[:], po[:], bl_t[:, :1])
                nc.sync.dma_start(out[:, :], ot[:])
    nc.compile()
    return nc


# ------------------------------------------------------------------- helpers
def _shard(batch):
    """Contiguous graph ranges balanced by node count."""
    cnt = np.bincount(batch, minlength=N_GRAPHS)
    csum = np.concatenate([[0], np.cumsum(cnt)])
    targets = np.linspace(0, N, N_CORES + 1)
    gcut = [0]
    for c in range(1, N_CORES):
        gcut.append(int(np.searchsorted(csum, targets[c])))
    gcut.append(N_GRAPHS)
    gcut = np.array(gcut)
    nbase = csum[gcut]  # node range per core
    return cnt, gcut, nbase


def kernel(x, edge_index, batch, W1, a_src1, a_dst1, b1,
           W2, a_src2, a_dst2, b2, Wlin, blin):
    x = np.asarray(x, np.float32)
    ei = np.asarray(edge_index, np.int64)
    batch = np.asarray(batch, np.int64)
    W1, a_src1, a_dst1, b1 = (np.asarray(a, np.float32)
                              for a in (W1, a_src1, a_dst1, b1))
    W2, a_src2, a_dst2, b2 = (np.asarray(a, np.float32)
                              for a in (W2, a_src2, a_dst2, b2))
    Wlin, blin = np.asarray(Wlin, np.float32), np.asarray(blin, np.float32)

    loops = np.arange(N, dtype=np.int64)
    src = np.concatenate([ei[0], loops]).astype(np.int32)
    dst = np.concatenate([ei[1], loops]).astype(np.int32)

    gcnt, gcut, nbase = _shard(batch)
    nodes = nbase[1:] - nbase[:-1]
    nodes_pad = int(-(-nodes.max() // P) * P)
    n_tiles = nodes_pad // P

    core_of_node = np.searchsorted(nbase[1:], np.arange(N), side="right")
    ecore = core_of_node[dst]
    dloc = dst - nbase[ecore]           # dst local node id
    etile = dloc // P                   # dst tile per edge

    # per (core, tile) counts -> uniform block structure
    cnt_ct = np.zeros((N_CORES, n_tiles), np.int64)
    np.add.at(cnt_ct, (ecore, etile), 1)
    b_uni = np.maximum(1, -(-cnt_ct.max(axis=0) // P))
    TB = int(b_uni.sum())
    cpre = np.concatenate([[0], np.cumsum(b_uni)]).astype(np.int64)

    # slot position of every edge: (partition, column)
    order = np.lexsort((etile, ecore))
    s_src, s_dloc, s_core, s_tile = (src[order], dloc[order], ecore[order],
                                     etile[order])
    # rank within (core, tile)
    key = s_core * n_tiles + s_tile
    start = np.searchsorted(key, np.arange(N_CORES * n_tiles), side="left")
    rank = np.arange(len(key)) - start[key]
    col = cpre[s_tile] + rank // P
    part = rank % P

    src_perm = np.zeros((N_CORES, P, TB), np.int32)
    dst_perm = np.zeros((N_CORES, P, TB), np.int32)
    dl_arr = np.full((N_CORES, P, TB), 200.0, np.float32)
    src_perm[s_core, part, col] = s_src
    dst_perm[s_core, part, col] = s_dloc + nbase[s_core]
    dl_arr[s_core, part, col] = (s_dloc % P).astype(np.float32)

    sig = (nodes_pad, tuple(b_uni.tolist()))
    if sig not in _cache:
        _cache[sig] = (build_A(nodes_pad),
                       build_edge(n_tiles, b_uni, False, nodes_pad),
                       build_edge(n_tiles, b_uni, True, nodes_pad))
    ncA, ncB, ncC = _cache[sig]

    iota = np.broadcast_to(np.arange(P, dtype=np.float32),
                           (P, P)).astype(ml_dtypes.bfloat16)
    cores = list(range(N_CORES))

    # ---- launch A
    w1aug = np.concatenate([W1, (W1 @ a_src1)[:, None],
                            (W1 @ a_dst1)[:, None]], axis=1).astype(np.float32)
    inA = []
    for c in cores:
        xT = np.zeros((P, nodes_pad), np.float32)
        xT[:, : nodes[c]] = x[nbase[c]:nbase[c + 1]].T
        inA.append({"xT": xT, "w1aug": w1aug})
    LAST_LAUNCH_WALLS.clear()
    resA = _run(ncA, inA, cores)
    h1 = np.empty((N, F_HID), np.float32)
    as1 = np.empty(N, np.float32)
    ad1 = np.empty(N, np.float32)
    for c in cores:
        n1 = resA.results[c]["node1"]
        h1[nbase[c]:nbase[c + 1]] = n1[:F_HID, : nodes[c]].T
        as1[nbase[c]:nbase[c + 1]] = n1[F_HID, : nodes[c]]
        ad1[nbase[c]:nbase[c + 1]] = n1[F_HID + 1, : nodes[c]]

    # ---- launch B
    def edge_streams(h, a_s, a_d):
        hb = h.astype(ml_dtypes.bfloat16)
        one = np.ones((P, TB, 1), ml_dtypes.bfloat16)
        hes, zss = [], []
        for c in cores:
            sp = src_perm[c]
            he = np.concatenate([one, hb[sp]], axis=2).reshape(
                P, TB * (F_HID + 1))
            z = a_s[sp] + a_d[dst_perm[c]]
            hes.append(he)
            zss.append(z.astype(np.float32))
        return hes, zss

    hes, zss = edge_streams(h1, as1, ad1)
    w2aug = np.concatenate([W2, (W2 @ a_src2)[:, None],
                            (W2 @ a_dst2)[:, None]], axis=1).astype(np.float32)
    b1rep = np.broadcast_to(b1, (P, F_HID)).astype(np.float32).copy()
    inB = [{"h_edges": hes[c], "z": zss[c], "dst_local": dl_arr[c],
            "iota": iota, "b_rep": b1rep, "w2aug": w2aug} for c in cores]
    resB = _run(ncB, inB, cores)
    h2 = np.empty((N, F_OUT), np.float32)
    as2 = np.empty(N, np.float32)
    ad2 = np.empty(N, np.float32)
    for c in cores:
        n2 = resB.results[c]["node2"]
        h2[nbase[c]:nbase[c + 1]] = n2[:F_OUT, : nodes[c]].T
        as2[nbase[c]:nbase[c + 1]] = n2[F_OUT, : nodes[c]]
        ad2[nbase[c]:nbase[c + 1]] = n2[F_OUT + 1, : nodes[c]]

    # ---- launch C
    hes2, zss2 = edge_streams(h2, as2, ad2)
    b2rep = np.broadcast_to(b2, (P, F_OUT)).astype(np.float32).copy()
    inC = []
    gid = batch.astype(np.int64)
    for c in cores:
        ng = gcut[c + 1] - gcut[c]
        pw = np.zeros((n_tiles, P, G_SLOTS), np.float32)
        gl = gid[nbase[c]:nbase[c + 1]] - gcut[c]  # local graph id per node
        nn = np.arange(nodes[c])
        pw[nn // P, nn % P, gl] = 1.0
        rc = np.ones((G_SLOTS, 1), np.float32)
        cc = gcnt[gcut[c]:gcut[c + 1]]
        rc[:ng, 0] = 1.0 / np.maximum(cc, 1.0)
        inC.append({"h_edges": hes2[c], "z": zss2[c], "dst_local": dl_arr[c],
                    "iota": iota, "b_rep": b2rep,
                    "poolw": pw.transpose(1, 0, 2).reshape(P,
                                                           n_tiles * G_SLOTS),
                    "rcnt": rc, "wlin": Wlin.astype(np.float32),
                    "blin": blin.reshape(N_CLS, 1).astype(np.float32)})
    resC = _run(ncC, inC, cores)
    out = np.empty((N_GRAPHS, N_CLS), np.float32)
    for c in cores:
        lg = resC.results[c]["logits"]
        ng = gcut[c + 1] - gcut[c]
        out[gcut[c]:gcut[c + 1]] = lg[:, :ng].T
    return out



# revision 20
# speedup vs baseline: 8.7790x; 8.7790x over previous
"""GAT (2-layer) + mean-pool + linear head on 8 Trainium2 NeuronCores.

Single-launch design (vs. the previous 3-launch host-gather version):
  - Nodes/graphs are split into 8 contiguous ranges balanced by node count
    (batch is sorted); each core owns its graphs' dst-nodes and the edges
    targeting them (data parallel over graphs, per the sharding hint).
  - Each core computes node features ([h | h@a_src | h@a_dst] via an
    augmented weight matmul) for its own shard, packs them into 256-byte
    per-node records in device DRAM, and the 8 shards are exchanged with an
    on-device AllGather (no host round trip).
  - Per-edge h[src]/a_src[src] are fetched on-device with gpsimd dma_gather
    from the AllGathered record table (two index streams, since gather
    indices are int16: rows < 32768 and the rest). a_dst[dst] and the
    dst-local one-hot key are fetched with a second gather from the core's
    own-shard table.
  - The segment softmax + weighted aggregation per 128-dst tile is a chain
    of one-hot scatter matmuls accumulated in PSUM; self-loop terms (PyG
    GATConv adds them) are applied analytically in the tile epilogue.
  - Mean-pool is a membership one-hot matmul; the linear head runs on-core;
    only [n_cls, 128] logits per core are downloaded.
  Host->device traffic is ~18 MB total (x in bf16 + int16 edge index
  streams) instead of ~280 MB of pre-gathered edge records.
"""

import sys

sys.path.insert(0, "/opt/trn_rl_repo")

import numpy as np
import ml_dtypes

import concourse.bacc as bacc
import concourse.mybir as mybir
import concourse.tile as tile
from concourse import bass_utils, library_config
from concourse.masks import make_identity

F32 = mybir.dt.float32
BF16 = mybir.dt.bfloat16
I16 = mybir.dt.int16

P = 128
NC = 8
NEG_SLOPE = 0.2
EPS = 1e-16
REC = 128          # bf16 elems per node record = 256B
# record layout (bf16 cols): 0:64 h, 64 one, 65 pad, 66:68 as(f32),
# 68:70 ad(f32), 70:72 lid(f32), 72:128 pad
FC_AS, FC_AD, FC_LID = 33, 34, 35   # f32-view columns
LID_PAD = 200.0
T0_DEFAULT = 32768
GRP_OVERRIDE = 0
MAXB = 8           # dma_gather deadlocks above 1024 idxs/call -> <=8 blocks

_cache = {}
LAST_LAUNCH_WALLS = []


def _run(nc, in_maps, cores):
    import time
    t0 = time.time()
    res = bass_utils.run_bass_kernel_spmd(nc, in_maps, core_ids=cores)
    LAST_LAUNCH_WALLS.append(time.time() - t0)
    return res


def build_gat(n_tiles, blo, bhi, f_dim, n_cls, t0_split):
    """One SPMD program for all 8 cores.

    n_tiles: dst tiles per core; blo/bhi: per-tile block counts for the
    low/high gather index streams; f_dim: hidden size (64); t0_split: row
    where the global record table is split for int16 gather indices.
    """
    F = f_dim
    FA = F + 2
    nodes_pad = n_tiles * P
    rows_my = (n_tiles + 1) * P        # + sentinel pad tile
    rows_g = NC * rows_my
    lo_rows = min(t0_split, rows_g)
    hi_rows = rows_g - lo_rows
    assert hi_rows <= 32768
    TBlo, TBhi = int(np.sum(blo)), int(np.sum(bhi))
    TB = TBlo + TBhi
    clo = np.concatenate([[0], np.cumsum(blo)]).astype(int)
    chi = np.concatenate([[0], np.cumsum(bhi)]).astype(int)

    GRP = GRP_OVERRIDE if GRP_OVERRIDE else (4 if n_tiles >= 4 else 1)
    groups = [(g, min(g + GRP, n_tiles)) for g in range(0, n_tiles, GRP)]
    max_lo = max(clo[t1] - clo[t0] for t0, t1 in groups)
    max_hi = max((chi[t1] - chi[t0] for t0, t1 in groups), default=0)

    nc = bacc.Bacc("TRN2", target_bir_lowering=False, debug=False,
                   num_devices=NC)
    xT = nc.dram_tensor("xT", [P, nodes_pad], BF16, kind="ExternalInput").ap()
    w1 = nc.dram_tensor("w1", [P, FA], BF16, kind="ExternalInput").ap()
    w2 = nc.dram_tensor("w2", [F, FA], BF16, kind="ExternalInput").ap()
    b1r = nc.dram_tensor("b1r", [P, F], F32, kind="ExternalInput").ap()
    b2r = nc.dram_tensor("b2r", [P, F], F32, kind="ExternalInput").ap()
    wl = nc.dram_tensor("wl", [F, n_cls], BF16, kind="ExternalInput").ap()
    bl = nc.dram_tensor("bl", [n_cls, 1], F32, kind="ExternalInput").ap()
    iota_in = nc.dram_tensor("iota", [P, P], BF16, kind="ExternalInput").ap()
    pcol_in = nc.dram_tensor("pcol", [P, 1], F32, kind="ExternalInput").ap()
    gsl = nc.dram_tensor("gsl", [16, TBlo * 8], I16, kind="ExternalInput").ap()
    gdl = nc.dram_tensor("gdl", [16, TBlo * 8], I16, kind="ExternalInput").ap()
    if TBhi:
        gsh = nc.dram_tensor("gsh", [16, TBhi * 8], I16,
                             kind="ExternalInput").ap()
        gdh = nc.dram_tensor("gdh", [16, TBhi * 8], I16,
                             kind="ExternalInput").ap()
    gl_in = nc.dram_tensor("gl", [P, n_tiles], F32, kind="ExternalInput").ap()
    rcnt = nc.dram_tensor("rcnt", [P, 1], F32, kind="ExternalInput").ap()
    out = nc.dram_tensor("logits", [n_cls, P], F32, kind="ExternalOutput").ap()

    with tile.TileContext(nc) as tc:
        with (
            tc.tile_pool(name="cst", bufs=1) as cst,
            tc.tile_pool(name="big", bufs=1) as big,
            tc.tile_pool(name="glo", bufs=2) as glop,
            tc.tile_pool(name="ghi", bufs=2) as ghip,
            tc.tile_pool(name="oh", bufs=6) as ohp,
            tc.tile_pool(name="sb", bufs=3) as sb,
            tc.tile_pool(name="zz", bufs=3) as zz,
            tc.tile_pool(name="dram", bufs=1, space="DRAM") as dram,
            tc.tile_pool(name="pacc", bufs=2, space="PSUM") as pacc,
            tc.tile_pool(name="pmm", bufs=2, space="PSUM") as pmm,
            tc.tile_pool(name="ptp", bufs=2, space="PSUM") as ptp,
            tc.tile_pool(name="ppl", bufs=1, space="PSUM") as ppl,
        ):
            # ---------------- constants / inputs to SBUF
            iota = cst.tile([P, P], BF16)
            nc.sync.dma_start(iota[:], iota_in[:, :])
            pcol = cst.tile([P, 1], F32)
            nc.sync.dma_start(pcol[:], pcol_in[:, :])
            ident = cst.tile([P, P], F32)
            make_identity(nc, ident[:])
            nc.gpsimd.load_library(library_config.mlp)
            w1s = cst.tile([P, FA], BF16)
            nc.sync.dma_start(w1s[:], w1[:, :])
            w2s = cst.tile([F, FA], BF16)
            nc.sync.dma_start(w2s[:], w2[:, :])
            b1s = cst.tile([P, F], F32)
            nc.sync.dma_start(b1s[:], b1r[:, :])
            b2s = cst.tile([P, F], F32)
            nc.sync.dma_start(b2s[:], b2r[:, :])
            wls = cst.tile([F, n_cls], BF16)
            nc.sync.dma_start(wls[:], wl[:, :])
            bls = cst.tile([n_cls, 1], F32)
            nc.sync.dma_start(bls[:], bl[:, :])
            gls = cst.tile([P, n_tiles], F32)
            nc.sync.dma_start(gls[:], gl_in[:, :])
            rcs = cst.tile([P, 1], F32)
            nc.sync.dma_start(rcs[:], rcnt[:, :])
            zcol = cst.tile([P, n_tiles], F32)
            nc.vector.memset(zcol[:], 0.0)

            # index streams, replicated into each 16-partition group
            isl = cst.tile([P, TBlo * 8], I16)
            idl = cst.tile([P, TBlo * 8], I16)
            for g in range(8):
                nc.sync.dma_start(isl[16 * g:16 * (g + 1), :], gsl[:, :])
                nc.sync.dma_start(idl[16 * g:16 * (g + 1), :], gdl[:, :])
            if TBhi:
                ish = cst.tile([P, TBhi * 8], I16)
                idh = cst.tile([P, TBhi * 8], I16)
                for g in range(8):
                    nc.sync.dma_start(ish[16 * g:16 * (g + 1), :], gsh[:, :])
                    nc.sync.dma_start(idh[16 * g:16 * (g + 1), :], gdh[:, :])

            xs = big.tile([P, nodes_pad], BF16)
            nc.sync.dma_start(xs[:], xT[:, :])

            # persistent per-layer state
            n1 = big.tile([FA, nodes_pad], F32)
            rec1 = big.tile([P, n_tiles * REC], BF16)
            rec2 = big.tile([P, n_tiles * REC], BF16)
            adA = big.tile([P, max(TB, 1)], F32)
            dlA = big.tile([P, max(TB, 1)], F32)
            elA = big.tile([P, max(TB, 1)], F32)
            esl = big.tile([P, n_tiles], F32)
            asc = big.tile([P, n_tiles], F32)
            adc = big.tile([P, n_tiles], F32)
            padrec = cst.tile([P, REC], BF16)
            nc.vector.memset(padrec[:], 0.0)
            nc.vector.memset(padrec[:].bitcast(F32)[:, FC_LID:FC_LID + 1],
                             LID_PAD)

            mytab = [dram.tile([rows_my, REC], BF16, name=f"mytab{i}")
                     for i in range(2)]
            gtab = [dram.tile([rows_g, REC], BF16, name=f"gtab{i}")
                    for i in range(2)]

            def rec_static(rec):
                rf = rec[:].bitcast(F32)
                nc.vector.memset(rec[:].rearrange(
                    "p (t e) -> p t e", e=REC)[:, :, 64:66], 0.0)
                nc.vector.memset(rec[:].rearrange(
                    "p (t e) -> p t e", e=REC)[:, :, 64:65], 1.0)
                nc.vector.memset(rf.rearrange(
                    "p (t e) -> p t e", e=REC // 2)[:, :, 36:64], 0.0)
                nc.vector.tensor_scalar_add(
                    rf[:, FC_LID::REC // 2], zcol[:], pcol[:, :1])

            def build_rec(rec, tp, t):
                """tp: PSUM [P, FA] node-major tile t -> record tile."""
                rf = rec[:].bitcast(F32)
                nc.scalar.copy(rec[:, t * REC:t * REC + F], tp[:, 0:F])
                nc.scalar.copy(
                    rf[:, t * (REC // 2) + FC_AS:t * (REC // 2) + FC_AS + 1],
                    tp[:, F:F + 1])
                nc.scalar.copy(
                    rf[:, t * (REC // 2) + FC_AD:t * (REC // 2) + FC_AD + 1],
                    tp[:, F + 1:F + 2])

            def finish_layer_tab(li, rec):
                """rec -> own-shard DRAM table (+ pad tile), AllGather, and
                node-aligned as/ad columns + self-loop factors."""
                tabv = mytab[li][0:n_tiles * P, :].rearrange(
                    "(p t) e -> p (t e)", p=P)
                nc.sync.dma_start(tabv, rec[:])
                nc.sync.dma_start(
                    mytab[li][n_tiles * P:(n_tiles + 1) * P, :], padrec[:])
                nc.gpsimd.collective_compute(
                    "AllGather", mybir.AluOpType.bypass,
                    replica_groups=[list(range(NC))],
                    ins=[mytab[li].opt()], outs=[gtab[li].opt()],
                )
                rf = rec[:].bitcast(F32)
                nc.scalar.copy(asc[:], rf[:, FC_AS::REC // 2])
                nc.scalar.copy(adc[:], rf[:, FC_AD::REC // 2])
                t1 = zz.tile([P, n_tiles], F32, tag="z1")
                nc.vector.tensor_tensor(out=t1[:], in0=asc[:], in1=adc[:],
                                        op=mybir.AluOpType.add)
                t2 = zz.tile([P, n_tiles], F32, tag="z2")
                nc.vector.tensor_scalar_mul(t2[:], t1[:], NEG_SLOPE)
                nc.vector.tensor_tensor(out=t1[:], in0=t1[:], in1=t2[:],
                                        op=mybir.AluOpType.max)
                nc.scalar.activation(esl[:], t1[:],
                                     mybir.ActivationFunctionType.Exp)

            def dst_gathers(li, need_dl):
                """ad (and layer-1: dst one-hot key) per edge slot from the
                own-shard table."""
                for t0g, t1g in groups:
                    for half in range(2):
                        if half == 0:
                            nb = clo[t1g] - clo[t0g]
                            cbase, idx, pool, mx = clo[t0g], idl, glop, max_lo
                            off = 0
                        else:
                            if not TBhi:
                                continue
                            nb = chi[t1g] - chi[t0g]
                            cbase, idx, pool, mx = chi[t0g], idh, ghip, max_hi
                            off = TBlo
                        if nb == 0:
                            continue
                        gt = pool.tile([P, mx * REC], BF16, tag=f"d{half}")
                        g3 = gt[:].rearrange("p (b e) -> p b e", e=REC)
                        for b0 in range(0, nb, MAXB):
                            b1 = min(b0 + MAXB, nb)
                            nc.gpsimd.dma_gather(
                                g3[:, b0:b1, :], mytab[li][:, :],
                                idx[:, (cbase + b0) * 8:(cbase + b1) * 8],
                                (b1 - b0) * P, (b1 - b0) * P, REC)
                        gf = gt[:].bitcast(F32)
                        c0, c1 = off + cbase, off + cbase + nb
                        nc.scalar.copy(
                            adA[:, c0:c1],
                            gf[:, FC_AD::REC // 2][:, :nb])
                        if need_dl:
                            nc.scalar.copy(
                                dlA[:, c0:c1],
                                gf[:, FC_LID::REC // 2][:, :nb])

            def src_gather_el(li, t0g, t1g, half):
                """Gather [h|1|as] records for a group's slots; compute el."""
                if half == 1 and not TBhi:
                    return None
                if half == 0:
                    nb = clo[t1g] - clo[t0g]
                    cbase, idx, pool, mx = clo[t0g], isl, glop, max_lo
                    off = 0
                    srct = gtab[li][0:lo_rows, :]
                else:
                    nb = chi[t1g] - chi[t0g]
                    cbase, idx, pool, mx = chi[t0g], ish, ghip, max_hi
                    off = TBlo
                    srct = gtab[li][lo_rows:rows_g, :]
                if nb == 0:
                    return None
                gt = pool.tile([P, mx * REC], BF16, tag=f"s{half}")
                g3 = gt[:].rearrange("p (b e) -> p b e", e=REC)
                for b0 in range(0, nb, MAXB):
                    b1 = min(b0 + MAXB, nb)
                    nc.gpsimd.dma_gather(
                        g3[:, b0:b1, :], srct,
                        idx[:, (cbase + b0) * 8:(cbase + b1) * 8],
                        (b1 - b0) * P, (b1 - b0) * P, REC)
                c0, c1 = off + cbase, off + cbase + nb
                gf = gt[:].bitcast(F32)
                zt = zz.tile([P, max(max_lo, max_hi)], F32, tag="ze")
                nc.vector.tensor_tensor(
                    out=zt[:, :nb], in0=gf[:, FC_AS::REC // 2][:, :nb],
                    in1=adA[:, c0:c1], op=mybir.AluOpType.add)
                z2 = zz.tile([P, max(max_lo, max_hi)], F32, tag="z2e")
                nc.vector.tensor_scalar_mul(z2[:, :nb], zt[:, :nb], NEG_SLOPE)
                nc.vector.tensor_tensor(out=zt[:, :nb], in0=zt[:, :nb],
                                        in1=z2[:, :nb],
                                        op=mybir.AluOpType.max)
                nc.scalar.activation(elA[:, c0:c1], zt[:, :nb],
                                     mybir.ActivationFunctionType.Exp)
                return gt

            def scatter_tile(t, t0g, gtl, gth):
                """Accumulate this dst tile's blocks into PSUM [P, F+1]."""
                acc = pacc.tile([P, F + 1], F32, tag="acc")
                work = []
                for j in range(clo[t + 1] - clo[t]):
                    work.append((gtl, j + clo[t] - clo[t0g], clo[t] + j))
                for j in range(chi[t + 1] - chi[t]):
                    work.append((gth, j + chi[t] - chi[t0g],
                                 TBlo + chi[t] + j))
                for k, (gt, brel, col) in enumerate(work):
                    oh = ohp.tile([P, P], BF16, tag="oh")
                    nc.vector.tensor_scalar(
                        oh[:], iota[:], dlA[:, col:col + 1],
                        elA[:, col:col + 1],
                        mybir.AluOpType.is_equal, mybir.AluOpType.mult)
                    nc.tensor.matmul(
                        acc[:], lhsT=oh[:],
                        rhs=gt[:, brel * REC:brel * REC + F + 1],
                        start=(k == 0), stop=(k == len(work) - 1))
                if not work:
                    accs = sb.tile([P, F + 1], F32, tag="acc0")
                    nc.vector.memset(accs[:], 0.0)
                    return accs
                return acc

            def epilogue(t, acc, rec):
                """Softmax-normalize + self-loop + bias -> [P, F] f32."""
                hsl = sb.tile([P, F], F32, tag="hsl")
                nc.vector.tensor_scalar_mul(
                    hsl[:], rec[:, t * REC:t * REC + F], esl[:, t:t + 1])
                num = sb.tile([P, F], F32, tag="num")
                nc.vector.tensor_tensor(out=num[:], in0=acc[:, 0:F],
                                        in1=hsl[:], op=mybir.AluOpType.add)
                den = sb.tile([P, 1], F32, tag="den")
                nc.vector.tensor_tensor(out=den[:], in0=acc[:, F:F + 1],
                                        in1=esl[:, t:t + 1],
                                        op=mybir.AluOpType.add)
                nc.vector.tensor_scalar_add(den[:], den[:], EPS)
                nc.vector.reciprocal(den[:], den[:])
                o = sb.tile([P, F], F32, tag="o")
                nc.vector.tensor_scalar_mul(o[:], num[:], den[:, :1])
                return o

            # ================= layer 1 node phase
            CH = 512
            for c0 in range(0, nodes_pad, CH):
                c1 = min(c0 + CH, nodes_pad)
                ps = pmm.tile([FA, 512], F32, tag="mm")
                nc.tensor.matmul(ps[:, :c1 - c0], lhsT=w1s[:],
                                 rhs=xs[:, c0:c1], start=True, stop=True)
                nc.scalar.copy(n1[:, c0:c1], ps[:, :c1 - c0])
            rec_static(rec1)
            for t in range(n_tiles):
                tp = ptp.tile([P, P], F32, tag="tp")
                nc.tensor.transpose(tp[:, :FA], n1[:, t * P:(t + 1) * P],
                                    ident[:FA, :FA])
                build_rec(rec1, tp, t)
            finish_layer_tab(0, rec1)
            dst_gathers(0, need_dl=True)

            # ================= layer 1 edges + layer 2 node phase
            rec_static(rec2)
            for t0g, t1g in groups:
                gtl = src_gather_el(0, t0g, t1g, 0)
                gth = src_gather_el(0, t0g, t1g, 1)
                for t in range(t0g, t1g):
                    acc = scatter_tile(t, t0g, gtl, gth)
                    o = epilogue(t, acc, rec1)
                    nc.vector.tensor_tensor(out=o[:], in0=o[:], in1=b1s[:],
                                            op=mybir.AluOpType.add)
                    nc.scalar.activation(o[:], o[:],
                                         mybir.ActivationFunctionType.Relu)
                    # layer-2 node compute for this tile
                    oT = ptp.tile([P, P], F32, tag="tp")
                    nc.tensor.transpose(oT[:F, :], o[:], ident[:])
                    hTb = sb.tile([F, P], BF16, tag="hTb")
                    nc.scalar.copy(hTb[:], oT[:F, :])
                    pnf = pmm.tile([FA, 512], F32, tag="mm")
                    pn = pnf[:, :P]
                    nc.tensor.matmul(pn, lhsT=w2s[:], rhs=hTb[:],
                                     start=True, stop=True)
                    n2s = sb.tile([FA, P], F32, tag="n2s")
                    nc.scalar.copy(n2s[:], pn)
                    tp2 = ptp.tile([P, P], F32, tag="tp")
                    nc.tensor.transpose(tp2[:, :FA], n2s[:], ident[:FA, :FA])
                    build_rec(rec2, tp2, t)
            finish_layer_tab(1, rec2)
            dst_gathers(1, need_dl=False)

            # ================= layer 2 edges + pooling
            pool_ps = ppl.tile([P, F], F32)
            for t0g, t1g in groups:
                gtl = src_gather_el(1, t0g, t1g, 0)
                gth = src_gather_el(1, t0g, t1g, 1)
                for t in range(t0g, t1g):
                    acc = scatter_tile(t, t0g, gtl, gth)
                    o = epilogue(t, acc, rec2)
                    nc.vector.tensor_tensor(out=o[:], in0=o[:], in1=b2s[:],
                                            op=mybir.AluOpType.add)
                    ob = sb.tile([P, F], BF16, tag="ob")
                    nc.vector.tensor_copy(ob[:], o[:])
                    ohg = ohp.tile([P, P], BF16, tag="ohg")
                    nc.vector.tensor_scalar(
                        ohg[:], iota[:], gls[:, t:t + 1], None,
                        mybir.AluOpType.is_equal)
                    nc.tensor.matmul(pool_ps[:], lhsT=ohg[:], rhs=ob[:],
                                     start=(t == 0), stop=(t == n_tiles - 1))

            # ================= head
            if True:
                pm = sb.tile([P, F], F32, tag="pm")
                nc.vector.tensor_scalar_mul(pm[:], pool_ps[:], rcs[:, :1])
                pT = ptp.tile([P, P], F32, tag="tp")
                nc.tensor.transpose(pT[:F, :], pm[:], ident[:])
                pTb = sb.tile([F, P], BF16, tag="pTb")
                nc.scalar.copy(pTb[:], pT[:F, :])
                pof = pmm.tile([FA, 512], F32, tag="mm")
                po = pof[:n_cls, :P]
                nc.tensor.matmul(po, lhsT=wls[:], rhs=pTb[:],
                                 start=True, stop=True)
                ot = sb.tile([n_cls, P], F32, tag="ot")
                nc.vector.tensor_scalar_add(ot[:], po, bls[:, :1])
                nc.sync.dma_start(out[:, :], ot[:])
    nc.compile()
    return nc


# ------------------------------------------------------------------- host
def _shard(batch, n, n_graphs):
    cnt = np.bincount(batch, minlength=n_graphs)
    csum = np.concatenate([[0], np.cumsum(cnt)])
    targets = np.linspace(0, n, NC + 1)
    gcut = [0]
    for c in range(1, NC):
        gcut.append(int(np.searchsorted(csum, targets[c])))
    gcut.append(n_graphs)
    gcut = np.array(gcut)
    nbase = csum[gcut]
    return cnt, gcut, nbase


def _wrap16(vals):
    """[n] -> [16, n/16] gather-index layout (position i -> [i%16, i//16])."""
    return np.ascontiguousarray(vals.reshape(-1, 16).T)


def kernel(x, edge_index, batch, W1, a_src1, a_dst1, b1,
           W2, a_src2, a_dst2, b2, Wlin, blin, t0_split=T0_DEFAULT):
    x = np.asarray(x, np.float32)
    ei = np.asarray(edge_index, np.int64)
    batch = np.asarray(batch, np.int64)
    W1, a_src1, a_dst1, b1 = (np.asarray(a, np.float32)
                              for a in (W1, a_src1, a_dst1, b1))
    W2, a_src2, a_dst2, b2 = (np.asarray(a, np.float32)
                              for a in (W2, a_src2, a_dst2, b2))
    Wlin, blin = np.asarray(Wlin, np.float32), np.asarray(blin, np.float32)

    N = x.shape[0]
    F = W1.shape[1]
    n_cls = Wlin.shape[1]
    n_graphs = int(batch.max()) + 1 if batch.size else 1
    src = ei[0].astype(np.int64)
    dst = ei[1].astype(np.int64)

    gcnt, gcut, nbase = _shard(batch, N, n_graphs)
    nodes = nbase[1:] - nbase[:-1]
    nodes_pad = int(-(-nodes.max() // P) * P)
    n_tiles = nodes_pad // P
    rows_my = (n_tiles + 1) * P
    assert (gcut[1:] - gcut[:-1]).max() <= P, "graphs per core must fit 128"

    core_of_node = np.searchsorted(nbase[1:], np.arange(N), side="right")
    # interleaved table row: node local nl -> (nl % P) * n_tiles + nl // P
    nloc_src = src - nbase[core_of_node[src]]
    srow = (core_of_node[src] * rows_my + (nloc_src % P) * n_tiles
            + nloc_src // P)
    ecore = core_of_node[dst]
    dloc = dst - nbase[ecore]
    et = dloc // P
    drow = (dloc % P) * n_tiles + dloc // P      # core-local table row
    half = (srow >= t0_split).astype(np.int64)

    key = (ecore * n_tiles + et) * 2 + half
    order = np.argsort(key, kind="stable")
    sk = key[order]
    starts = np.searchsorted(sk, np.arange(NC * n_tiles * 2))
    rank = np.arange(len(sk)) - starts[sk]

    cnt_cth = np.bincount(key, minlength=NC * n_tiles * 2).reshape(
        NC, n_tiles, 2)
    bmax = (-(-cnt_cth // P)).max(axis=0)        # ceil, then max over cores
    blo, bhi = bmax[:, 0], bmax[:, 1]
    TBlo, TBhi = int(blo.sum()), int(bhi.sum())
    clo = np.concatenate([[0], np.cumsum(blo)]).astype(np.int64)
    chi = np.concatenate([[0], np.cumsum(bhi)]).astype(np.int64)

    s_src, s_dst = srow[order], drow[order]
    s_core, s_t, s_h = ecore[order], et[order], half[order]
    colh = np.where(s_h == 0, clo[s_t], chi[s_t]) + rank // P
    pos = colh * P + rank % P

    slo_a = np.zeros((NC, TBlo * P), np.int16)
    dlo_a = np.full((NC, TBlo * P), n_tiles * P, np.int16)  # pad -> sentinel
    shi_a = np.zeros((NC, max(TBhi, 1) * P), np.int16)
    dhi_a = np.full((NC, max(TBhi, 1) * P), n_tiles * P, np.int16)
    m0 = s_h == 0
    slo_a[s_core[m0], pos[m0]] = s_src[m0]
    dlo_a[s_core[m0], pos[m0]] = s_dst[m0]
    m1 = s_h == 1
    shi_a[s_core[m1], pos[m1]] = s_src[m1] - t0_split
    dhi_a[s_core[m1], pos[m1]] = s_dst[m1]

    sig = (n_tiles, tuple(blo.tolist()), tuple(bhi.tolist()), F, n_cls,
           t0_split)
    if sig not in _cache:
        _cache[sig] = build_gat(n_tiles, blo, bhi, F, n_cls, t0_split)
    ncm = _cache[sig]

    w1aug = np.concatenate([W1, (W1 @ a_src1)[:, None],
                            (W1 @ a_dst1)[:, None]], axis=1)
    w2aug = np.concatenate([W2, (W2 @ a_src2)[:, None],
                            (W2 @ a_dst2)[:, None]], axis=1)
    iota = np.broadcast_to(np.arange(P, dtype=np.float32), (P, P))
    common = {
        "w1": w1aug.astype(ml_dtypes.bfloat16),
        "w2": w2aug.astype(ml_dtypes.bfloat16),
        "b1r": np.broadcast_to(b1, (P, F)).astype(np.float32).copy(),
        "b2r": np.broadcast_to(b2, (P, F)).astype(np.float32).copy(),
        "wl": Wlin.astype(ml_dtypes.bfloat16),
        "bl": blin.reshape(n_cls, 1).astype(np.float32),
        "iota": iota.astype(ml_dtypes.bfloat16),
        "pcol": np.arange(P, dtype=np.float32).reshape(P, 1),
    }
    in_maps = []
    gid = batch.astype(np.int64)
    for c in range(NC):
        xTc = np.zeros((P, nodes_pad), np.float32)
        xTc[:, :nodes[c]] = x[nbase[c]:nbase[c + 1]].T
        glc = np.full((P, n_tiles), LID_PAD, np.float32)
        nn = np.arange(nodes[c])
        glc[nn % P, nn // P] = gid[nbase[c]:nbase[c + 1]] - gcut[c]
        rc = np.ones((P, 1), np.float32)
        ng = gcut[c + 1] - gcut[c]
        rc[:ng, 0] = 1.0 / np.maximum(gcnt[gcut[c]:gcut[c + 1]], 1.0)
        m = {
            "xT": xTc.astype(ml_dtypes.bfloat16),
            "gsl": _wrap16(slo_a[c]), "gdl": _wrap16(dlo_a[c]),
            "gl": glc, "rcnt": rc,
        }
        if TBhi:
            m["gsh"] = _wrap16(shi_a[c])
            m["gdh"] = _wrap16(dhi_a[c])
        m.update(common)
        in_maps.append(m)

    LAST_LAUNCH_WALLS.clear()
    res = _run(ncm, in_maps, list(range(NC)))
    out = np.empty((n_graphs, n_cls), np.float32)
    for c in range(NC):
        lg = res.results[c]["logits"]
        ng = gcut[c + 1] - gcut[c]
        out[gcut[c]:gcut[c + 1]] = lg[:, :ng].T
    return out


# revision 21
# speedup vs baseline: 10.6418x; 1.2122x over previous
"""GAT (2-layer) + mean-pool + linear head on 8 Trainium2 NeuronCores.

Single-launch design (vs. the previous 3-launch host-gather version):
  - Nodes/graphs are split into 8 contiguous ranges balanced by node count
    (batch is sorted); each core owns its graphs' dst-nodes and the edges
    targeting them (data parallel over graphs, per the sharding hint).
  - Each core computes node features ([h | h@a_src | h@a_dst] via an
    augmented weight matmul) for its own shard, packs them into 256-byte
    per-node records in device DRAM, and the 8 shards are exchanged with an
    on-device AllGather (no host round trip).
  - Per-edge h[src]/a_src[src] are fetched on-device with gpsimd dma_gather
    from the AllGathered record table (two index streams, since gather
    indices are int16: rows < 32768 and the rest). a_dst[dst] and the
    dst-local one-hot key are fetched with a second gather from the core's
    own-shard table.
  - The segment softmax + weighted aggregation per 128-dst tile is a chain
    of one-hot scatter matmuls accumulated in PSUM; self-loop terms (PyG
    GATConv adds them) are applied analytically in the tile epilogue.
  - Mean-pool is a membership one-hot matmul; the linear head runs on-core;
    only [n_cls, 128] logits per core are downloaded.
  Host->device traffic is ~18 MB total (x in bf16 + int16 edge index
  streams) instead of ~280 MB of pre-gathered edge records.
"""

import sys

sys.path.insert(0, "/opt/trn_rl_repo")

import numpy as np
import ml_dtypes

import concourse.bacc as bacc
import concourse.mybir as mybir
import concourse.tile as tile
from concourse import bass_utils, library_config
from concourse.masks import make_identity

F32 = mybir.dt.float32
BF16 = mybir.dt.bfloat16
F8 = mybir.dt.float8e4
I16 = mybir.dt.int16

P = 128
NC = 8
NEG_SLOPE = 0.2
EPS = 1e-16
REC = 128          # bf16 elems per node record = 256B
# record layout (bf16 cols): 0:64 h, 64 one, 65 pad, 66:68 as(f32),
# 68:70 ad(f32), 70:72 lid(f32), 72:128 pad
FC_AS, FC_AD, FC_LID = 33, 34, 35   # f32-view columns
LID_PAD = 200.0
T0_DEFAULT = 32768
GRP_OVERRIDE = 0
MAXB = 8           # dma_gather deadlocks above 1024 idxs/call -> <=8 blocks

_cache = {}
LAST_LAUNCH_WALLS = []


def _run(nc, in_maps, cores):
    import time
    t0 = time.time()
    res = bass_utils.run_bass_kernel_spmd(nc, in_maps, core_ids=cores)
    LAST_LAUNCH_WALLS.append(time.time() - t0)
    return res


def build_gat(n_tiles, blo, bhi, f_dim, n_cls, t0_split):
    """One SPMD program for all 8 cores.

    n_tiles: dst tiles per core; blo/bhi: per-tile block counts for the
    low/high gather index streams; f_dim: hidden size (64); t0_split: row
    where the global record table is split for int16 gather indices.
    """
    F = f_dim
    FA = F + 2
    nodes_pad = n_tiles * P
    rows_my = (n_tiles + 1) * P        # + sentinel pad tile
    rows_g = NC * rows_my
    lo_rows = min(t0_split, rows_g)
    hi_rows = rows_g - lo_rows
    assert hi_rows <= 32768
    TBlo, TBhi = int(np.sum(blo)), int(np.sum(bhi))
    TB = TBlo + TBhi
    clo = np.concatenate([[0], np.cumsum(blo)]).astype(int)
    chi = np.concatenate([[0], np.cumsum(bhi)]).astype(int)

    GRP = GRP_OVERRIDE if GRP_OVERRIDE else (4 if n_tiles >= 4 else 1)
    groups = [(g, min(g + GRP, n_tiles)) for g in range(0, n_tiles, GRP)]
    max_lo = max(clo[t1] - clo[t0] for t0, t1 in groups)
    max_hi = max((chi[t1] - chi[t0] for t0, t1 in groups), default=0)

    nc = bacc.Bacc("TRN2", target_bir_lowering=False, debug=False,
                   num_devices=NC)
    xT = nc.dram_tensor("xT", [P, nodes_pad], F8, kind="ExternalInput").ap()
    w1 = nc.dram_tensor("w1", [P, FA], BF16, kind="ExternalInput").ap()
    w2 = nc.dram_tensor("w2", [F, FA], BF16, kind="ExternalInput").ap()
    b1r = nc.dram_tensor("b1r", [P, F], F32, kind="ExternalInput").ap()
    b2r = nc.dram_tensor("b2r", [P, F], F32, kind="ExternalInput").ap()
    wl = nc.dram_tensor("wl", [F, n_cls], BF16, kind="ExternalInput").ap()
    bl = nc.dram_tensor("bl", [n_cls, 1], F32, kind="ExternalInput").ap()
    iota_in = nc.dram_tensor("iota", [P, P], BF16, kind="ExternalInput").ap()
    pcol_in = nc.dram_tensor("pcol", [P, 1], F32, kind="ExternalInput").ap()
    gsl = nc.dram_tensor("gsl", [16, TBlo * 8], I16, kind="ExternalInput").ap()
    gdl = nc.dram_tensor("gdl", [16, TBlo * 8], I16, kind="ExternalInput").ap()
    if TBhi:
        gsh = nc.dram_tensor("gsh", [16, TBhi * 8], I16,
                             kind="ExternalInput").ap()
        gdh = nc.dram_tensor("gdh", [16, TBhi * 8], I16,
                             kind="ExternalInput").ap()
    gl_in = nc.dram_tensor("gl", [P, n_tiles], F32, kind="ExternalInput").ap()
    rcnt = nc.dram_tensor("rcnt", [P, 1], F32, kind="ExternalInput").ap()
    out = nc.dram_tensor("logits", [n_cls, P], F32, kind="ExternalOutput").ap()

    with tile.TileContext(nc) as tc:
        with (
            tc.tile_pool(name="cst", bufs=1) as cst,
            tc.tile_pool(name="big", bufs=1) as big,
            tc.tile_pool(name="glo", bufs=2) as glop,
            tc.tile_pool(name="ghi", bufs=2) as ghip,
            tc.tile_pool(name="oh", bufs=6) as ohp,
            tc.tile_pool(name="sb", bufs=3) as sb,
            tc.tile_pool(name="zz", bufs=3) as zz,
            tc.tile_pool(name="dram", bufs=1, space="DRAM") as dram,
            tc.tile_pool(name="pacc", bufs=2, space="PSUM") as pacc,
            tc.tile_pool(name="pmm", bufs=2, space="PSUM") as pmm,
            tc.tile_pool(name="ptp", bufs=2, space="PSUM") as ptp,
            tc.tile_pool(name="ppl", bufs=1, space="PSUM") as ppl,
        ):
            # ---------------- constants / inputs to SBUF
            iota = cst.tile([P, P], BF16)
            nc.sync.dma_start(iota[:], iota_in[:, :])
            pcol = cst.tile([P, 1], F32)
            nc.sync.dma_start(pcol[:], pcol_in[:, :])
            ident = cst.tile([P, P], F32)
            make_identity(nc, ident[:])
            nc.gpsimd.load_library(library_config.mlp)
            w1s = cst.tile([P, FA], BF16)
            nc.sync.dma_start(w1s[:], w1[:, :])
            w2s = cst.tile([F, FA], BF16)
            nc.sync.dma_start(w2s[:], w2[:, :])
            b1s = cst.tile([P, F], F32)
            nc.sync.dma_start(b1s[:], b1r[:, :])
            b2s = cst.tile([P, F], F32)
            nc.sync.dma_start(b2s[:], b2r[:, :])
            wls = cst.tile([F, n_cls], BF16)
            nc.sync.dma_start(wls[:], wl[:, :])
            bls = cst.tile([n_cls, 1], F32)
            nc.sync.dma_start(bls[:], bl[:, :])
            gls = cst.tile([P, n_tiles], F32)
            nc.sync.dma_start(gls[:], gl_in[:, :])
            rcs = cst.tile([P, 1], F32)
            nc.sync.dma_start(rcs[:], rcnt[:, :])
            zcol = cst.tile([P, n_tiles], F32)
            nc.vector.memset(zcol[:], 0.0)

            # index streams, replicated into each 16-partition group
            isl = cst.tile([P, TBlo * 8], I16)
            idl = cst.tile([P, TBlo * 8], I16)
            for g in range(8):
                nc.sync.dma_start(isl[16 * g:16 * (g + 1), :], gsl[:, :])
                nc.sync.dma_start(idl[16 * g:16 * (g + 1), :], gdl[:, :])
            if TBhi:
                ish = cst.tile([P, TBhi * 8], I16)
                idh = cst.tile([P, TBhi * 8], I16)
                for g in range(8):
                    nc.sync.dma_start(ish[16 * g:16 * (g + 1), :], gsh[:, :])
                    nc.sync.dma_start(idh[16 * g:16 * (g + 1), :], gdh[:, :])

            xs8 = big.tile([P, nodes_pad], F8)
            nc.sync.dma_start(xs8[:], xT[:, :])
            xs = big.tile([P, nodes_pad], BF16)
            nc.vector.tensor_copy(xs[:], xs8[:])

            # persistent per-layer state
            n1 = big.tile([FA, nodes_pad], F32)
            rec1 = big.tile([P, n_tiles * REC], BF16)
            rec2 = big.tile([P, n_tiles * REC], BF16)
            adA = big.tile([P, max(TB, 1)], F32)
            dlA = big.tile([P, max(TB, 1)], F32)
            elA = big.tile([P, max(TB, 1)], F32)
            esl = big.tile([P, n_tiles], F32)
            asc = big.tile([P, n_tiles], F32)
            adc = big.tile([P, n_tiles], F32)
            padrec = cst.tile([P, REC], BF16)
            nc.vector.memset(padrec[:], 0.0)
            nc.vector.memset(padrec[:].bitcast(F32)[:, FC_LID:FC_LID + 1],
                             LID_PAD)

            mytab = [dram.tile([rows_my, REC], BF16, name=f"mytab{i}")
                     for i in range(2)]
            gtab = [dram.tile([rows_g, REC], BF16, name=f"gtab{i}")
                    for i in range(2)]

            def rec_static(rec):
                rf = rec[:].bitcast(F32)
                nc.vector.memset(rec[:].rearrange(
                    "p (t e) -> p t e", e=REC)[:, :, 64:66], 0.0)
                nc.vector.memset(rec[:].rearrange(
                    "p (t e) -> p t e", e=REC)[:, :, 64:65], 1.0)
                nc.vector.memset(rf.rearrange(
                    "p (t e) -> p t e", e=REC // 2)[:, :, 36:64], 0.0)
                nc.vector.tensor_scalar_add(
                    rf[:, FC_LID::REC // 2], zcol[:], pcol[:, :1])

            def build_rec(rec, tp, t):
                """tp: PSUM [P, FA] node-major tile t -> record tile."""
                rf = rec[:].bitcast(F32)
                nc.scalar.copy(rec[:, t * REC:t * REC + F], tp[:, 0:F])
                nc.scalar.copy(
                    rf[:, t * (REC // 2) + FC_AS:t * (REC // 2) + FC_AS + 1],
                    tp[:, F:F + 1])
                nc.scalar.copy(
                    rf[:, t * (REC // 2) + FC_AD:t * (REC // 2) + FC_AD + 1],
                    tp[:, F + 1:F + 2])

            def finish_layer_tab(li, rec):
                """rec -> own-shard DRAM table (+ pad tile), AllGather, and
                node-aligned as/ad columns + self-loop factors."""
                tabv = mytab[li][0:n_tiles * P, :].rearrange(
                    "(p t) e -> p (t e)", p=P)
                nc.sync.dma_start(tabv, rec[:])
                nc.sync.dma_start(
                    mytab[li][n_tiles * P:(n_tiles + 1) * P, :], padrec[:])
                nc.gpsimd.collective_compute(
                    "AllGather", mybir.AluOpType.bypass,
                    replica_groups=[list(range(NC))],
                    ins=[mytab[li].opt()], outs=[gtab[li].opt()],
                )
                rf = rec[:].bitcast(F32)
                nc.scalar.copy(asc[:], rf[:, FC_AS::REC // 2])
                nc.scalar.copy(adc[:], rf[:, FC_AD::REC // 2])
                t1 = zz.tile([P, n_tiles], F32, tag="z1")
                nc.vector.tensor_tensor(out=t1[:], in0=asc[:], in1=adc[:],
                                        op=mybir.AluOpType.add)
                t2 = zz.tile([P, n_tiles], F32, tag="z2")
                nc.vector.tensor_scalar_mul(t2[:], t1[:], NEG_SLOPE)
                nc.vector.tensor_tensor(out=t1[:], in0=t1[:], in1=t2[:],
                                        op=mybir.AluOpType.max)
                nc.scalar.activation(esl[:], t1[:],
                                     mybir.ActivationFunctionType.Exp)

            def dst_gathers(li, need_dl):
                """ad (and layer-1: dst one-hot key) per edge slot from the
                own-shard table."""
                for t0g, t1g in groups:
                    for half in range(2):
                        if half == 0:
                            nb = clo[t1g] - clo[t0g]
                            cbase, idx, pool, mx = clo[t0g], idl, glop, max_lo
                            off = 0
                        else:
                            if not TBhi:
                                continue
                            nb = chi[t1g] - chi[t0g]
                            cbase, idx, pool, mx = chi[t0g], idh, ghip, max_hi
                            off = TBlo
                        if nb == 0:
                            continue
                        gt = pool.tile([P, mx * REC], BF16, tag=f"d{half}")
                        g3 = gt[:].rearrange("p (b e) -> p b e", e=REC)
                        for b0 in range(0, nb, MAXB):
                            b1 = min(b0 + MAXB, nb)
                            nc.gpsimd.dma_gather(
                                g3[:, b0:b1, :], mytab[li][:, :],
                                idx[:, (cbase + b0) * 8:(cbase + b1) * 8],
                                (b1 - b0) * P, (b1 - b0) * P, REC)
                        gf = gt[:].bitcast(F32)
                        c0, c1 = off + cbase, off + cbase + nb
                        nc.scalar.copy(
                            adA[:, c0:c1],
                            gf[:, FC_AD::REC // 2][:, :nb])
                        if need_dl:
                            nc.scalar.copy(
                                dlA[:, c0:c1],
                                gf[:, FC_LID::REC // 2][:, :nb])

            def src_gather_el(li, t0g, t1g, half):
                """Gather [h|1|as] records for a group's slots; compute el."""
                if half == 1 and not TBhi:
                    return None
                if half == 0:
                    nb = clo[t1g] - clo[t0g]
                    cbase, idx, pool, mx = clo[t0g], isl, glop, max_lo
                    off = 0
                    srct = gtab[li][0:lo_rows, :]
                else:
                    nb = chi[t1g] - chi[t0g]
                    cbase, idx, pool, mx = chi[t0g], ish, ghip, max_hi
                    off = TBlo
                    srct = gtab[li][lo_rows:rows_g, :]
                if nb == 0:
                    return None
                gt = pool.tile([P, mx * REC], BF16, tag=f"s{half}")
                g3 = gt[:].rearrange("p (b e) -> p b e", e=REC)
                for b0 in range(0, nb, MAXB):
                    b1 = min(b0 + MAXB, nb)
                    nc.gpsimd.dma_gather(
                        g3[:, b0:b1, :], srct,
                        idx[:, (cbase + b0) * 8:(cbase + b1) * 8],
                        (b1 - b0) * P, (b1 - b0) * P, REC)
                c0, c1 = off + cbase, off + cbase + nb
                gf = gt[:].bitcast(F32)
                zt = zz.tile([P, max(max_lo, max_hi)], F32, tag="ze")
                nc.vector.tensor_tensor(
                    out=zt[:, :nb], in0=gf[:, FC_AS::REC // 2][:, :nb],
                    in1=adA[:, c0:c1], op=mybir.AluOpType.add)
                z2 = zz.tile([P, max(max_lo, max_hi)], F32, tag="z2e")
                nc.vector.tensor_scalar_mul(z2[:, :nb], zt[:, :nb], NEG_SLOPE)
                nc.vector.tensor_tensor(out=zt[:, :nb], in0=zt[:, :nb],
                                        in1=z2[:, :nb],
                                        op=mybir.AluOpType.max)
                nc.scalar.activation(elA[:, c0:c1], zt[:, :nb],
                                     mybir.ActivationFunctionType.Exp)
                return gt

            def scatter_tile(t, t0g, gtl, gth):
                """Accumulate this dst tile's blocks into PSUM [P, F+1]."""
                acc = pacc.tile([P, F + 1], F32, tag="acc")
                work = []
                for j in range(clo[t + 1] - clo[t]):
                    work.append((gtl, j + clo[t] - clo[t0g], clo[t] + j))
                for j in range(chi[t + 1] - chi[t]):
                    work.append((gth, j + chi[t] - chi[t0g],
                                 TBlo + chi[t] + j))
                for k, (gt, brel, col) in enumerate(work):
                    oh = ohp.tile([P, P], BF16, tag="oh")
                    nc.vector.tensor_scalar(
                        oh[:], iota[:], dlA[:, col:col + 1],
                        elA[:, col:col + 1],
                        mybir.AluOpType.is_equal, mybir.AluOpType.mult)
                    nc.tensor.matmul(
                        acc[:], lhsT=oh[:],
                        rhs=gt[:, brel * REC:brel * REC + F + 1],
                        start=(k == 0), stop=(k == len(work) - 1))
                if not work:
                    accs = sb.tile([P, F + 1], F32, tag="acc0")
                    nc.vector.memset(accs[:], 0.0)
                    return accs
                return acc

            def epilogue(t, acc, rec):
                """Softmax-normalize + self-loop + bias -> [P, F] f32."""
                hsl = sb.tile([P, F], F32, tag="hsl")
                nc.vector.tensor_scalar_mul(
                    hsl[:], rec[:, t * REC:t * REC + F], esl[:, t:t + 1])
                num = sb.tile([P, F], F32, tag="num")
                nc.vector.tensor_tensor(out=num[:], in0=acc[:, 0:F],
                                        in1=hsl[:], op=mybir.AluOpType.add)
                den = sb.tile([P, 1], F32, tag="den")
                nc.vector.tensor_tensor(out=den[:], in0=acc[:, F:F + 1],
                                        in1=esl[:, t:t + 1],
                                        op=mybir.AluOpType.add)
                nc.vector.tensor_scalar_add(den[:], den[:], EPS)
                nc.vector.reciprocal(den[:], den[:])
                o = sb.tile([P, F], F32, tag="o")
                nc.vector.tensor_scalar_mul(o[:], num[:], den[:, :1])
                return o

            # ================= layer 1 node phase
            CH = 512
            for c0 in range(0, nodes_pad, CH):
                c1 = min(c0 + CH, nodes_pad)
                ps = pmm.tile([FA, 512], F32, tag="mm")
                nc.tensor.matmul(ps[:, :c1 - c0], lhsT=w1s[:],
                                 rhs=xs[:, c0:c1], start=True, stop=True)
                nc.scalar.copy(n1[:, c0:c1], ps[:, :c1 - c0])
            rec_static(rec1)
            for t in range(n_tiles):
                tp = ptp.tile([P, P], F32, tag="tp")
                nc.tensor.transpose(tp[:, :FA], n1[:, t * P:(t + 1) * P],
                                    ident[:FA, :FA])
                build_rec(rec1, tp, t)
            finish_layer_tab(0, rec1)
            dst_gathers(0, need_dl=True)

            # ================= layer 1 edges + layer 2 node phase
            rec_static(rec2)
            for t0g, t1g in groups:
                gtl = src_gather_el(0, t0g, t1g, 0)
                gth = src_gather_el(0, t0g, t1g, 1)
                for t in range(t0g, t1g):
                    acc = scatter_tile(t, t0g, gtl, gth)
                    o = epilogue(t, acc, rec1)
                    nc.vector.tensor_tensor(out=o[:], in0=o[:], in1=b1s[:],
                                            op=mybir.AluOpType.add)
                    nc.scalar.activation(o[:], o[:],
                                         mybir.ActivationFunctionType.Relu)
                    # layer-2 node compute for this tile
                    oT = ptp.tile([P, P], F32, tag="tp")
                    nc.tensor.transpose(oT[:F, :], o[:], ident[:])
                    hTb = sb.tile([F, P], BF16, tag="hTb")
                    nc.scalar.copy(hTb[:], oT[:F, :])
                    pnf = pmm.tile([FA, 512], F32, tag="mm")
                    pn = pnf[:, :P]
                    nc.tensor.matmul(pn, lhsT=w2s[:], rhs=hTb[:],
                                     start=True, stop=True)
                    n2s = sb.tile([FA, P], F32, tag="n2s")
                    nc.scalar.copy(n2s[:], pn)
                    tp2 = ptp.tile([P, P], F32, tag="tp")
                    nc.tensor.transpose(tp2[:, :FA], n2s[:], ident[:FA, :FA])
                    build_rec(rec2, tp2, t)
            finish_layer_tab(1, rec2)
            dst_gathers(1, need_dl=False)

            # ================= layer 2 edges + pooling
            pool_ps = ppl.tile([P, F], F32)
            for t0g, t1g in groups:
                gtl = src_gather_el(1, t0g, t1g, 0)
                gth = src_gather_el(1, t0g, t1g, 1)
                for t in range(t0g, t1g):
                    acc = scatter_tile(t, t0g, gtl, gth)
                    o = epilogue(t, acc, rec2)
                    nc.vector.tensor_tensor(out=o[:], in0=o[:], in1=b2s[:],
                                            op=mybir.AluOpType.add)
                    ob = sb.tile([P, F], BF16, tag="ob")
                    nc.vector.tensor_copy(ob[:], o[:])
                    ohg = ohp.tile([P, P], BF16, tag="ohg")
                    nc.vector.tensor_scalar(
                        ohg[:], iota[:], gls[:, t:t + 1], None,
                        mybir.AluOpType.is_equal)
                    nc.tensor.matmul(pool_ps[:], lhsT=ohg[:], rhs=ob[:],
                                     start=(t == 0), stop=(t == n_tiles - 1))

            # ================= head
            if True:
                pm = sb.tile([P, F], F32, tag="pm")
                nc.vector.tensor_scalar_mul(pm[:], pool_ps[:], rcs[:, :1])
                pT = ptp.tile([P, P], F32, tag="tp")
                nc.tensor.transpose(pT[:F, :], pm[:], ident[:])
                pTb = sb.tile([F, P], BF16, tag="pTb")
                nc.scalar.copy(pTb[:], pT[:F, :])
                pof = pmm.tile([FA, 512], F32, tag="mm")
                po = pof[:n_cls, :P]
                nc.tensor.matmul(po, lhsT=wls[:], rhs=pTb[:],
                                 start=True, stop=True)
                ot = sb.tile([n_cls, P], F32, tag="ot")
                nc.vector.tensor_scalar_add(ot[:], po, bls[:, :1])
                nc.sync.dma_start(out[:, :], ot[:])
    nc.compile()
    return nc


# ------------------------------------------------------------------- host
def _shard(batch, n, n_graphs):
    cnt = np.bincount(batch, minlength=n_graphs)
    csum = np.concatenate([[0], np.cumsum(cnt)])
    targets = np.linspace(0, n, NC + 1)
    gcut = [0]
    for c in range(1, NC):
        gcut.append(int(np.searchsorted(csum, targets[c])))
    gcut.append(n_graphs)
    gcut = np.array(gcut)
    nbase = csum[gcut]
    return cnt, gcut, nbase


def _wrap16(vals):
    """[n] -> [16, n/16] gather-index layout (position i -> [i%16, i//16])."""
    return np.ascontiguousarray(vals.reshape(-1, 16).T)


def kernel(x, edge_index, batch, W1, a_src1, a_dst1, b1,
           W2, a_src2, a_dst2, b2, Wlin, blin, t0_split=T0_DEFAULT):
    x = np.asarray(x, np.float32)
    ei = np.asarray(edge_index, np.int64)
    batch = np.asarray(batch, np.int64)
    W1, a_src1, a_dst1, b1 = (np.asarray(a, np.float32)
                              for a in (W1, a_src1, a_dst1, b1))
    W2, a_src2, a_dst2, b2 = (np.asarray(a, np.float32)
                              for a in (W2, a_src2, a_dst2, b2))
    Wlin, blin = np.asarray(Wlin, np.float32), np.asarray(blin, np.float32)

    N = x.shape[0]
    F = W1.shape[1]
    n_cls = Wlin.shape[1]
    n_graphs = int(batch.max()) + 1 if batch.size else 1
    src = ei[0].astype(np.int64)
    dst = ei[1].astype(np.int64)

    gcnt, gcut, nbase = _shard(batch, N, n_graphs)
    nodes = nbase[1:] - nbase[:-1]
    nodes_pad = int(-(-nodes.max() // P) * P)
    n_tiles = nodes_pad // P
    rows_my = (n_tiles + 1) * P
    assert (gcut[1:] - gcut[:-1]).max() <= P, "graphs per core must fit 128"

    core_of_node = np.searchsorted(nbase[1:], np.arange(N), side="right")
    # interleaved table row: node local nl -> (nl % P) * n_tiles + nl // P
    nloc_src = src - nbase[core_of_node[src]]
    srow = (core_of_node[src] * rows_my + (nloc_src % P) * n_tiles
            + nloc_src // P)
    ecore = core_of_node[dst]
    dloc = dst - nbase[ecore]
    et = dloc // P
    drow = (dloc % P) * n_tiles + dloc // P      # core-local table row
    half = (srow >= t0_split).astype(np.int64)

    key = (ecore * n_tiles + et) * 2 + half
    order = np.argsort(key, kind="stable")
    sk = key[order]
    starts = np.searchsorted(sk, np.arange(NC * n_tiles * 2))
    rank = np.arange(len(sk)) - starts[sk]

    cnt_cth = np.bincount(key, minlength=NC * n_tiles * 2).reshape(
        NC, n_tiles, 2)
    bmax = (-(-cnt_cth // P)).max(axis=0)        # ceil, then max over cores
    blo, bhi = bmax[:, 0], bmax[:, 1]
    TBlo, TBhi = int(blo.sum()), int(bhi.sum())
    clo = np.concatenate([[0], np.cumsum(blo)]).astype(np.int64)
    chi = np.concatenate([[0], np.cumsum(bhi)]).astype(np.int64)

    s_src, s_dst = srow[order], drow[order]
    s_core, s_t, s_h = ecore[order], et[order], half[order]
    colh = np.where(s_h == 0, clo[s_t], chi[s_t]) + rank // P
    pos = colh * P + rank % P

    slo_a = np.zeros((NC, TBlo * P), np.int16)
    dlo_a = np.full((NC, TBlo * P), n_tiles * P, np.int16)  # pad -> sentinel
    shi_a = np.zeros((NC, max(TBhi, 1) * P), np.int16)
    dhi_a = np.full((NC, max(TBhi, 1) * P), n_tiles * P, np.int16)
    m0 = s_h == 0
    slo_a[s_core[m0], pos[m0]] = s_src[m0]
    dlo_a[s_core[m0], pos[m0]] = s_dst[m0]
    m1 = s_h == 1
    shi_a[s_core[m1], pos[m1]] = s_src[m1] - t0_split
    dhi_a[s_core[m1], pos[m1]] = s_dst[m1]

    sig = (n_tiles, tuple(blo.tolist()), tuple(bhi.tolist()), F, n_cls,
           t0_split)
    if sig not in _cache:
        _cache[sig] = build_gat(n_tiles, blo, bhi, F, n_cls, t0_split)
    ncm = _cache[sig]

    w1aug = np.concatenate([W1, (W1 @ a_src1)[:, None],
                            (W1 @ a_dst1)[:, None]], axis=1)
    w2aug = np.concatenate([W2, (W2 @ a_src2)[:, None],
                            (W2 @ a_dst2)[:, None]], axis=1)
    iota = np.broadcast_to(np.arange(P, dtype=np.float32), (P, P))
    common = {
        "w1": w1aug.astype(ml_dtypes.bfloat16),
        "w2": w2aug.astype(ml_dtypes.bfloat16),
        "b1r": np.broadcast_to(b1, (P, F)).astype(np.float32).copy(),
        "b2r": np.broadcast_to(b2, (P, F)).astype(np.float32).copy(),
        "wl": Wlin.astype(ml_dtypes.bfloat16),
        "bl": blin.reshape(n_cls, 1).astype(np.float32),
        "iota": iota.astype(ml_dtypes.bfloat16),
        "pcol": np.arange(P, dtype=np.float32).reshape(P, 1),
    }
    in_maps = []
    gid = batch.astype(np.int64)
    for c in range(NC):
        xTc = np.zeros((P, nodes_pad), np.float32)
        xTc[:, :nodes[c]] = x[nbase[c]:nbase[c + 1]].T
        glc = np.full((P, n_tiles), LID_PAD, np.float32)
        nn = np.arange(nodes[c])
        glc[nn % P, nn // P] = gid[nbase[c]:nbase[c + 1]] - gcut[c]
        rc = np.ones((P, 1), np.float32)
        ng = gcut[c + 1] - gcut[c]
        rc[:ng, 0] = 1.0 / np.maximum(gcnt[gcut[c]:gcut[c + 1]], 1.0)
        m = {
            "xT": xTc.astype(ml_dtypes.float8_e4m3),
            "gsl": _wrap16(slo_a[c]), "gdl": _wrap16(dlo_a[c]),
            "gl": glc, "rcnt": rc,
        }
        if TBhi:
            m["gsh"] = _wrap16(shi_a[c])
            m["gdh"] = _wrap16(dhi_a[c])
        m.update(common)
        in_maps.append(m)

    LAST_LAUNCH_WALLS.clear()
    res = _run(ncm, in_maps, list(range(NC)))
    out = np.empty((n_graphs, n_cls), np.float32)
    for c in range(NC):
        lg = res.results[c]["logits"]
        ng = gcut[c + 1] - gcut[c]
        out[gcut[c]:gcut[c + 1]] = lg[:, :ng].T
    return out


# revision 22
# speedup vs baseline: 21.4990x; 2.0202x over previous
"""GAT (2-layer) + mean-pool + linear head on 8 Trainium2 NeuronCores.

Single-launch design (vs. the previous 3-launch host-gather version):
  - Nodes/graphs are split into 8 contiguous ranges balanced by node count
    (batch is sorted); each core owns its graphs' dst-nodes and the edges
    targeting them (data parallel over graphs, per the sharding hint).
  - Each core computes node features ([h | h@a_src | h@a_dst] via an
    augmented weight matmul) for its own shard, packs them into 256-byte
    per-node records in device DRAM, and the 8 shards are exchanged with an
    on-device AllGather (no host round trip).
  - Per-edge h[src]/a_src[src] are fetched on-device with gpsimd dma_gather
    from the AllGathered record table (two index streams, since gather
    indices are int16: rows < 32768 and the rest). a_dst[dst] and the
    dst-local one-hot key are fetched with a second gather from the core's
    own-shard table.
  - The segment softmax + weighted aggregation per 128-dst tile is a chain
    of one-hot scatter matmuls accumulated in PSUM; self-loop terms (PyG
    GATConv adds them) are applied analytically in the tile epilogue.
  - Mean-pool is a membership one-hot matmul; the linear head runs on-core;
    only [n_cls, 128] logits per core are downloaded.
  Host->device traffic is ~18 MB total (x in bf16 + int16 edge index
  streams) instead of ~280 MB of pre-gathered edge records.
"""

import sys

sys.path.insert(0, "/opt/trn_rl_repo")

import numpy as np
import ml_dtypes

import jax

# Persistent XLA compilation cache: run_bass_kernel_spmd re-jits its wrapper
# on every call, so without this each launch pays ~0.7s of XLA re-compile.
jax.config.update("jax_compilation_cache_dir", "/tmp/jax_comp_cache")
jax.config.update("jax_persistent_cache_min_entry_size_bytes", -1)
jax.config.update("jax_persistent_cache_min_compile_time_secs", 0.0)

import concourse.bacc as bacc
import concourse.mybir as mybir
import concourse.tile as tile
from concourse import bass_utils, library_config
from concourse.masks import make_identity

F32 = mybir.dt.float32
BF16 = mybir.dt.bfloat16
F8 = mybir.dt.float8e4
I16 = mybir.dt.int16

P = 128
NC = 8
NEG_SLOPE = 0.2
EPS = 1e-16
REC = 128          # bf16 elems per node record = 256B
# record layout (bf16 cols): 0:64 h, 64 one, 65 pad, 66:68 as(f32),
# 68:70 ad(f32), 70:72 lid(f32), 72:128 pad
FC_AS, FC_AD, FC_LID = 33, 34, 35   # f32-view columns
LID_PAD = 200.0
T0_DEFAULT = 32768
GRP_OVERRIDE = 0
MAXB = 8           # dma_gather deadlocks above 1024 idxs/call -> <=8 blocks

_cache = {}
LAST_LAUNCH_WALLS = []


def _run(nc, in_maps, cores):
    import time
    t0 = time.time()
    res = bass_utils.run_bass_kernel_spmd(nc, in_maps, core_ids=cores)
    LAST_LAUNCH_WALLS.append(time.time() - t0)
    return res


def build_gat(n_tiles, blo, bhi, f_dim, n_cls, t0_split):
    """One SPMD program for all 8 cores.

    n_tiles: dst tiles per core; blo/bhi: per-tile block counts for the
    low/high gather index streams; f_dim: hidden size (64); t0_split: row
    where the global record table is split for int16 gather indices.
    """
    F = f_dim
    FA = F + 2
    nodes_pad = n_tiles * P
    rows_my = (n_tiles + 1) * P        # + sentinel pad tile
    rows_g = NC * rows_my
    lo_rows = min(t0_split, rows_g)
    hi_rows = rows_g - lo_rows
    assert hi_rows <= 32768
    TBlo, TBhi = int(np.sum(blo)), int(np.sum(bhi))
    TB = TBlo + TBhi
    clo = np.concatenate([[0], np.cumsum(blo)]).astype(int)
    chi = np.concatenate([[0], np.cumsum(bhi)]).astype(int)

    GRP = GRP_OVERRIDE if GRP_OVERRIDE else (4 if n_tiles >= 4 else 1)
    groups = [(g, min(g + GRP, n_tiles)) for g in range(0, n_tiles, GRP)]
    max_lo = max(clo[t1] - clo[t0] for t0, t1 in groups)
    max_hi = max((chi[t1] - chi[t0] for t0, t1 in groups), default=0)

    nc = bacc.Bacc("TRN2", target_bir_lowering=False, debug=False,
                   num_devices=NC)
    xT = nc.dram_tensor("xT", [P, nodes_pad], F8, kind="ExternalInput").ap()
    w1 = nc.dram_tensor("w1", [P, FA], BF16, kind="ExternalInput").ap()
    w2 = nc.dram_tensor("w2", [F, FA], BF16, kind="ExternalInput").ap()
    b1r = nc.dram_tensor("b1r", [P, F], F32, kind="ExternalInput").ap()
    b2r = nc.dram_tensor("b2r", [P, F], F32, kind="ExternalInput").ap()
    wl = nc.dram_tensor("wl", [F, n_cls], BF16, kind="ExternalInput").ap()
    bl = nc.dram_tensor("bl", [n_cls, 1], F32, kind="ExternalInput").ap()
    iota_in = nc.dram_tensor("iota", [P, P], BF16, kind="ExternalInput").ap()
    pcol_in = nc.dram_tensor("pcol", [P, 1], F32, kind="ExternalInput").ap()
    gsl = nc.dram_tensor("gsl", [16, TBlo * 8], I16, kind="ExternalInput").ap()
    gdl = nc.dram_tensor("gdl", [16, TBlo * 8], I16, kind="ExternalInput").ap()
    if TBhi:
        gsh = nc.dram_tensor("gsh", [16, TBhi * 8], I16,
                             kind="ExternalInput").ap()
        gdh = nc.dram_tensor("gdh", [16, TBhi * 8], I16,
                             kind="ExternalInput").ap()
    gl_in = nc.dram_tensor("gl", [P, n_tiles], F32, kind="ExternalInput").ap()
    rcnt = nc.dram_tensor("rcnt", [P, 1], F32, kind="ExternalInput").ap()
    out = nc.dram_tensor("logits", [n_cls, P], F32, kind="ExternalOutput").ap()

    with tile.TileContext(nc) as tc:
        with (
            tc.tile_pool(name="cst", bufs=1) as cst,
            tc.tile_pool(name="big", bufs=1) as big,
            tc.tile_pool(name="glo", bufs=2) as glop,
            tc.tile_pool(name="ghi", bufs=2) as ghip,
            tc.tile_pool(name="oh", bufs=6) as ohp,
            tc.tile_pool(name="sb", bufs=3) as sb,
            tc.tile_pool(name="zz", bufs=3) as zz,
            tc.tile_pool(name="dram", bufs=1, space="DRAM") as dram,
            tc.tile_pool(name="pacc", bufs=2, space="PSUM") as pacc,
            tc.tile_pool(name="pmm", bufs=2, space="PSUM") as pmm,
            tc.tile_pool(name="ptp", bufs=2, space="PSUM") as ptp,
            tc.tile_pool(name="ppl", bufs=1, space="PSUM") as ppl,
        ):
            # ---------------- constants / inputs to SBUF
            iota = cst.tile([P, P], BF16)
            nc.sync.dma_start(iota[:], iota_in[:, :])
            pcol = cst.tile([P, 1], F32)
            nc.sync.dma_start(pcol[:], pcol_in[:, :])
            ident = cst.tile([P, P], F32)
            make_identity(nc, ident[:])
            nc.gpsimd.load_library(library_config.mlp)
            w1s = cst.tile([P, FA], BF16)
            nc.sync.dma_start(w1s[:], w1[:, :])
            w2s = cst.tile([F, FA], BF16)
            nc.sync.dma_start(w2s[:], w2[:, :])
            b1s = cst.tile([P, F], F32)
            nc.sync.dma_start(b1s[:], b1r[:, :])
            b2s = cst.tile([P, F], F32)
            nc.sync.dma_start(b2s[:], b2r[:, :])
            wls = cst.tile([F, n_cls], BF16)
            nc.sync.dma_start(wls[:], wl[:, :])
            bls = cst.tile([n_cls, 1], F32)
            nc.sync.dma_start(bls[:], bl[:, :])
            gls = cst.tile([P, n_tiles], F32)
            nc.sync.dma_start(gls[:], gl_in[:, :])
            rcs = cst.tile([P, 1], F32)
            nc.sync.dma_start(rcs[:], rcnt[:, :])
            zcol = cst.tile([P, n_tiles], F32)
            nc.vector.memset(zcol[:], 0.0)

            # index streams, replicated into each 16-partition group
            isl = cst.tile([P, TBlo * 8], I16)
            idl = cst.tile([P, TBlo * 8], I16)
            for g in range(8):
                nc.sync.dma_start(isl[16 * g:16 * (g + 1), :], gsl[:, :])
                nc.sync.dma_start(idl[16 * g:16 * (g + 1), :], gdl[:, :])
            if TBhi:
                ish = cst.tile([P, TBhi * 8], I16)
                idh = cst.tile([P, TBhi * 8], I16)
                for g in range(8):
                    nc.sync.dma_start(ish[16 * g:16 * (g + 1), :], gsh[:, :])
                    nc.sync.dma_start(idh[16 * g:16 * (g + 1), :], gdh[:, :])

            xs8 = big.tile([P, nodes_pad], F8)
            nc.sync.dma_start(xs8[:], xT[:, :])
            xs = big.tile([P, nodes_pad], BF16)
            nc.vector.tensor_copy(xs[:], xs8[:])

            # persistent per-layer state
            n1 = big.tile([FA, nodes_pad], F32)
            rec1 = big.tile([P, n_tiles * REC], BF16)
            rec2 = big.tile([P, n_tiles * REC], BF16)
            adA = big.tile([P, max(TB, 1)], F32)
            dlA = big.tile([P, max(TB, 1)], F32)
            elA = big.tile([P, max(TB, 1)], F32)
            esl = big.tile([P, n_tiles], F32)
            asc = big.tile([P, n_tiles], F32)
            adc = big.tile([P, n_tiles], F32)
            padrec = cst.tile([P, REC], BF16)
            nc.vector.memset(padrec[:], 0.0)
            nc.vector.memset(padrec[:].bitcast(F32)[:, FC_LID:FC_LID + 1],
                             LID_PAD)

            mytab = [dram.tile([rows_my, REC], BF16, name=f"mytab{i}")
                     for i in range(2)]
            gtab = [dram.tile([rows_g, REC], BF16, name=f"gtab{i}")
                    for i in range(2)]

            def rec_static(rec):
                rf = rec[:].bitcast(F32)
                nc.vector.memset(rec[:].rearrange(
                    "p (t e) -> p t e", e=REC)[:, :, 64:66], 0.0)
                nc.vector.memset(rec[:].rearrange(
                    "p (t e) -> p t e", e=REC)[:, :, 64:65], 1.0)
                nc.vector.memset(rf.rearrange(
                    "p (t e) -> p t e", e=REC // 2)[:, :, 36:64], 0.0)
                nc.vector.tensor_scalar_add(
                    rf[:, FC_LID::REC // 2], zcol[:], pcol[:, :1])

            def build_rec(rec, tp, t):
                """tp: PSUM [P, FA] node-major tile t -> record tile."""
                rf = rec[:].bitcast(F32)
                nc.scalar.copy(rec[:, t * REC:t * REC + F], tp[:, 0:F])
                nc.scalar.copy(
                    rf[:, t * (REC // 2) + FC_AS:t * (REC // 2) + FC_AS + 1],
                    tp[:, F:F + 1])
                nc.scalar.copy(
                    rf[:, t * (REC // 2) + FC_AD:t * (REC // 2) + FC_AD + 1],
                    tp[:, F + 1:F + 2])

            def finish_layer_tab(li, rec):
                """rec -> own-shard DRAM table (+ pad tile), AllGather, and
                node-aligned as/ad columns + self-loop factors."""
                tabv = mytab[li][0:n_tiles * P, :].rearrange(
                    "(p t) e -> p (t e)", p=P)
                nc.sync.dma_start(tabv, rec[:])
                nc.sync.dma_start(
                    mytab[li][n_tiles * P:(n_tiles + 1) * P, :], padrec[:])
                nc.gpsimd.collective_compute(
                    "AllGather", mybir.AluOpType.bypass,
                    replica_groups=[list(range(NC))],
                    ins=[mytab[li].opt()], outs=[gtab[li].opt()],
                )
                rf = rec[:].bitcast(F32)
                nc.scalar.copy(asc[:], rf[:, FC_AS::REC // 2])
                nc.scalar.copy(adc[:], rf[:, FC_AD::REC // 2])
                t1 = zz.tile([P, n_tiles], F32, tag="z1")
                nc.vector.tensor_tensor(out=t1[:], in0=asc[:], in1=adc[:],
                                        op=mybir.AluOpType.add)
                t2 = zz.tile([P, n_tiles], F32, tag="z2")
                nc.vector.tensor_scalar_mul(t2[:], t1[:], NEG_SLOPE)
                nc.vector.tensor_tensor(out=t1[:], in0=t1[:], in1=t2[:],
                                        op=mybir.AluOpType.max)
                nc.scalar.activation(esl[:], t1[:],
                                     mybir.ActivationFunctionType.Exp)

            def dst_gathers(li, need_dl):
                """ad (and layer-1: dst one-hot key) per edge slot from the
                own-shard table."""
                for t0g, t1g in groups:
                    for half in range(2):
                        if half == 0:
                            nb = clo[t1g] - clo[t0g]
                            cbase, idx, pool, mx = clo[t0g], idl, glop, max_lo
                            off = 0
                        else:
                            if not TBhi:
                                continue
                            nb = chi[t1g] - chi[t0g]
                            cbase, idx, pool, mx = chi[t0g], idh, ghip, max_hi
                            off = TBlo
                        if nb == 0:
                            continue
                        gt = pool.tile([P, mx * REC], BF16, tag=f"d{half}")
                        g3 = gt[:].rearrange("p (b e) -> p b e", e=REC)
                        for b0 in range(0, nb, MAXB):
                            b1 = min(b0 + MAXB, nb)
                            nc.gpsimd.dma_gather(
                                g3[:, b0:b1, :], mytab[li][:, :],
                                idx[:, (cbase + b0) * 8:(cbase + b1) * 8],
                                (b1 - b0) * P, (b1 - b0) * P, REC)
                        gf = gt[:].bitcast(F32)
                        c0, c1 = off + cbase, off + cbase + nb
                        nc.scalar.copy(
                            adA[:, c0:c1],
                            gf[:, FC_AD::REC // 2][:, :nb])
                        if need_dl:
                            nc.scalar.copy(
                                dlA[:, c0:c1],
                                gf[:, FC_LID::REC // 2][:, :nb])

            def src_gather_el(li, t0g, t1g, half):
                """Gather [h|1|as] records for a group's slots; compute el."""
                if half == 1 and not TBhi:
                    return None
                if half == 0:
                    nb = clo[t1g] - clo[t0g]
                    cbase, idx, pool, mx = clo[t0g], isl, glop, max_lo
                    off = 0
                    srct = gtab[li][0:lo_rows, :]
                else:
                    nb = chi[t1g] - chi[t0g]
                    cbase, idx, pool, mx = chi[t0g], ish, ghip, max_hi
                    off = TBlo
                    srct = gtab[li][lo_rows:rows_g, :]
                if nb == 0:
                    return None
                gt = pool.tile([P, mx * REC], BF16, tag=f"s{half}")
                g3 = gt[:].rearrange("p (b e) -> p b e", e=REC)
                for b0 in range(0, nb, MAXB):
                    b1 = min(b0 + MAXB, nb)
                    nc.gpsimd.dma_gather(
                        g3[:, b0:b1, :], srct,
                        idx[:, (cbase + b0) * 8:(cbase + b1) * 8],
                        (b1 - b0) * P, (b1 - b0) * P, REC)
                c0, c1 = off + cbase, off + cbase + nb
                gf = gt[:].bitcast(F32)
                zt = zz.tile([P, max(max_lo, max_hi)], F32, tag="ze")
                nc.vector.tensor_tensor(
                    out=zt[:, :nb], in0=gf[:, FC_AS::REC // 2][:, :nb],
                    in1=adA[:, c0:c1], op=mybir.AluOpType.add)
                z2 = zz.tile([P, max(max_lo, max_hi)], F32, tag="z2e")
                nc.vector.tensor_scalar_mul(z2[:, :nb], zt[:, :nb], NEG_SLOPE)
                nc.vector.tensor_tensor(out=zt[:, :nb], in0=zt[:, :nb],
                                        in1=z2[:, :nb],
                                        op=mybir.AluOpType.max)
                nc.scalar.activation(elA[:, c0:c1], zt[:, :nb],
                                     mybir.ActivationFunctionType.Exp)
                return gt

            def scatter_tile(t, t0g, gtl, gth):
                """Accumulate this dst tile's blocks into PSUM [P, F+1]."""
                acc = pacc.tile([P, F + 1], F32, tag="acc")
                work = []
                for j in range(clo[t + 1] - clo[t]):
                    work.append((gtl, j + clo[t] - clo[t0g], clo[t] + j))
                for j in range(chi[t + 1] - chi[t]):
                    work.append((gth, j + chi[t] - chi[t0g],
                                 TBlo + chi[t] + j))
                for k, (gt, brel, col) in enumerate(work):
                    oh = ohp.tile([P, P], BF16, tag="oh")
                    nc.vector.tensor_scalar(
                        oh[:], iota[:], dlA[:, col:col + 1],
                        elA[:, col:col + 1],
                        mybir.AluOpType.is_equal, mybir.AluOpType.mult)
                    nc.tensor.matmul(
                        acc[:], lhsT=oh[:],
                        rhs=gt[:, brel * REC:brel * REC + F + 1],
                        start=(k == 0), stop=(k == len(work) - 1))
                if not work:
                    accs = sb.tile([P, F + 1], F32, tag="acc0")
                    nc.vector.memset(accs[:], 0.0)
                    return accs
                return acc

            def epilogue(t, acc, rec):
                """Softmax-normalize + self-loop + bias -> [P, F] f32."""
                hsl = sb.tile([P, F], F32, tag="hsl")
                nc.vector.tensor_scalar_mul(
                    hsl[:], rec[:, t * REC:t * REC + F], esl[:, t:t + 1])
                num = sb.tile([P, F], F32, tag="num")
                nc.vector.tensor_tensor(out=num[:], in0=acc[:, 0:F],
                                        in1=hsl[:], op=mybir.AluOpType.add)
                den = sb.tile([P, 1], F32, tag="den")
                nc.vector.tensor_tensor(out=den[:], in0=acc[:, F:F + 1],
                                        in1=esl[:, t:t + 1],
                                        op=mybir.AluOpType.add)
                nc.vector.tensor_scalar_add(den[:], den[:], EPS)
                nc.vector.reciprocal(den[:], den[:])
                o = sb.tile([P, F], F32, tag="o")
                nc.vector.tensor_scalar_mul(o[:], num[:], den[:, :1])
                return o

            # ================= layer 1 node phase
            CH = 512
            for c0 in range(0, nodes_pad, CH):
                c1 = min(c0 + CH, nodes_pad)
                ps = pmm.tile([FA, 512], F32, tag="mm")
                nc.tensor.matmul(ps[:, :c1 - c0], lhsT=w1s[:],
                                 rhs=xs[:, c0:c1], start=True, stop=True)
                nc.scalar.copy(n1[:, c0:c1], ps[:, :c1 - c0])
            rec_static(rec1)
            for t in range(n_tiles):
                tp = ptp.tile([P, P], F32, tag="tp")
                nc.tensor.transpose(tp[:, :FA], n1[:, t * P:(t + 1) * P],
                                    ident[:FA, :FA])
                build_rec(rec1, tp, t)
            finish_layer_tab(0, rec1)
            dst_gathers(0, need_dl=True)

            # ================= layer 1 edges + layer 2 node phase
            rec_static(rec2)
            for t0g, t1g in groups:
                gtl = src_gather_el(0, t0g, t1g, 0)
                gth = src_gather_el(0, t0g, t1g, 1)
                for t in range(t0g, t1g):
                    acc = scatter_tile(t, t0g, gtl, gth)
                    o = epilogue(t, acc, rec1)
                    nc.vector.tensor_tensor(out=o[:], in0=o[:], in1=b1s[:],
                                            op=mybir.AluOpType.add)
                    nc.scalar.activation(o[:], o[:],
                                         mybir.ActivationFunctionType.Relu)
                    # layer-2 node compute for this tile
                    oT = ptp.tile([P, P], F32, tag="tp")
                    nc.tensor.transpose(oT[:F, :], o[:], ident[:])
                    hTb = sb.tile([F, P], BF16, tag="hTb")
                    nc.scalar.copy(hTb[:], oT[:F, :])
                    pnf = pmm.tile([FA, 512], F32, tag="mm")
                    pn = pnf[:, :P]
                    nc.tensor.matmul(pn, lhsT=w2s[:], rhs=hTb[:],
                                     start=True, stop=True)
                    n2s = sb.tile([FA, P], F32, tag="n2s")
                    nc.scalar.copy(n2s[:], pn)
                    tp2 = ptp.tile([P, P], F32, tag="tp")
                    nc.tensor.transpose(tp2[:, :FA], n2s[:], ident[:FA, :FA])
                    build_rec(rec2, tp2, t)
            finish_layer_tab(1, rec2)
            dst_gathers(1, need_dl=False)

            # ================= layer 2 edges + pooling
            pool_ps = ppl.tile([P, F], F32)
            for t0g, t1g in groups:
                gtl = src_gather_el(1, t0g, t1g, 0)
                gth = src_gather_el(1, t0g, t1g, 1)
                for t in range(t0g, t1g):
                    acc = scatter_tile(t, t0g, gtl, gth)
                    o = epilogue(t, acc, rec2)
                    nc.vector.tensor_tensor(out=o[:], in0=o[:], in1=b2s[:],
                                            op=mybir.AluOpType.add)
                    ob = sb.tile([P, F], BF16, tag="ob")
                    nc.vector.tensor_copy(ob[:], o[:])
                    ohg = ohp.tile([P, P], BF16, tag="ohg")
                    nc.vector.tensor_scalar(
                        ohg[:], iota[:], gls[:, t:t + 1], None,
                        mybir.AluOpType.is_equal)
                    nc.tensor.matmul(pool_ps[:], lhsT=ohg[:], rhs=ob[:],
                                     start=(t == 0), stop=(t == n_tiles - 1))

            # ================= head
            if True:
                pm = sb.tile([P, F], F32, tag="pm")
                nc.vector.tensor_scalar_mul(pm[:], pool_ps[:], rcs[:, :1])
                pT = ptp.tile([P, P], F32, tag="tp")
                nc.tensor.transpose(pT[:F, :], pm[:], ident[:])
                pTb = sb.tile([F, P], BF16, tag="pTb")
                nc.scalar.copy(pTb[:], pT[:F, :])
                pof = pmm.tile([FA, 512], F32, tag="mm")
                po = pof[:n_cls, :P]
                nc.tensor.matmul(po, lhsT=wls[:], rhs=pTb[:],
                                 start=True, stop=True)
                ot = sb.tile([n_cls, P], F32, tag="ot")
                nc.vector.tensor_scalar_add(ot[:], po, bls[:, :1])
                nc.sync.dma_start(out[:, :], ot[:])
    nc.compile()
    return nc


# ------------------------------------------------------------------- host
def _shard(batch, n, n_graphs):
    cnt = np.bincount(batch, minlength=n_graphs)
    csum = np.concatenate([[0], np.cumsum(cnt)])
    targets = np.linspace(0, n, NC + 1)
    gcut = [0]
    for c in range(1, NC):
        gcut.append(int(np.searchsorted(csum, targets[c])))
    gcut.append(n_graphs)
    gcut = np.array(gcut)
    nbase = csum[gcut]
    return cnt, gcut, nbase


def _wrap16(vals):
    """[n] -> [16, n/16] gather-index layout (position i -> [i%16, i//16])."""
    return np.ascontiguousarray(vals.reshape(-1, 16).T)


def kernel(x, edge_index, batch, W1, a_src1, a_dst1, b1,
           W2, a_src2, a_dst2, b2, Wlin, blin, t0_split=T0_DEFAULT):
    x = np.asarray(x, np.float32)
    ei = np.asarray(edge_index, np.int64)
    batch = np.asarray(batch, np.int64)
    W1, a_src1, a_dst1, b1 = (np.asarray(a, np.float32)
                              for a in (W1, a_src1, a_dst1, b1))
    W2, a_src2, a_dst2, b2 = (np.asarray(a, np.float32)
                              for a in (W2, a_src2, a_dst2, b2))
    Wlin, blin = np.asarray(Wlin, np.float32), np.asarray(blin, np.float32)

    N = x.shape[0]
    F = W1.shape[1]
    n_cls = Wlin.shape[1]
    n_graphs = int(batch.max()) + 1 if batch.size else 1
    src = ei[0].astype(np.int64)
    dst = ei[1].astype(np.int64)

    gcnt, gcut, nbase = _shard(batch, N, n_graphs)
    nodes = nbase[1:] - nbase[:-1]
    nodes_pad = int(-(-nodes.max() // P) * P)
    n_tiles = nodes_pad // P
    rows_my = (n_tiles + 1) * P
    assert (gcut[1:] - gcut[:-1]).max() <= P, "graphs per core must fit 128"

    core_of_node = np.searchsorted(nbase[1:], np.arange(N), side="right")
    # interleaved table row: node local nl -> (nl % P) * n_tiles + nl // P
    nloc_src = src - nbase[core_of_node[src]]
    srow = (core_of_node[src] * rows_my + (nloc_src % P) * n_tiles
            + nloc_src // P)
    ecore = core_of_node[dst]
    dloc = dst - nbase[ecore]
    et = dloc // P
    drow = (dloc % P) * n_tiles + dloc // P      # core-local table row
    half = (srow >= t0_split).astype(np.int64)

    key = (ecore * n_tiles + et) * 2 + half
    order = np.argsort(key, kind="stable")
    sk = key[order]
    starts = np.searchsorted(sk, np.arange(NC * n_tiles * 2))
    rank = np.arange(len(sk)) - starts[sk]

    cnt_cth = np.bincount(key, minlength=NC * n_tiles * 2).reshape(
        NC, n_tiles, 2)
    bmax = (-(-cnt_cth // P)).max(axis=0)        # ceil, then max over cores
    blo, bhi = bmax[:, 0], bmax[:, 1]
    TBlo, TBhi = int(blo.sum()), int(bhi.sum())
    clo = np.concatenate([[0], np.cumsum(blo)]).astype(np.int64)
    chi = np.concatenate([[0], np.cumsum(bhi)]).astype(np.int64)

    s_src, s_dst = srow[order], drow[order]
    s_core, s_t, s_h = ecore[order], et[order], half[order]
    colh = np.where(s_h == 0, clo[s_t], chi[s_t]) + rank // P
    pos = colh * P + rank % P

    slo_a = np.zeros((NC, TBlo * P), np.int16)
    dlo_a = np.full((NC, TBlo * P), n_tiles * P, np.int16)  # pad -> sentinel
    shi_a = np.zeros((NC, max(TBhi, 1) * P), np.int16)
    dhi_a = np.full((NC, max(TBhi, 1) * P), n_tiles * P, np.int16)
    m0 = s_h == 0
    slo_a[s_core[m0], pos[m0]] = s_src[m0]
    dlo_a[s_core[m0], pos[m0]] = s_dst[m0]
    m1 = s_h == 1
    shi_a[s_core[m1], pos[m1]] = s_src[m1] - t0_split
    dhi_a[s_core[m1], pos[m1]] = s_dst[m1]

    sig = (n_tiles, tuple(blo.tolist()), tuple(bhi.tolist()), F, n_cls,
           t0_split)
    if sig not in _cache:
        _cache[sig] = build_gat(n_tiles, blo, bhi, F, n_cls, t0_split)
    ncm = _cache[sig]

    w1aug = np.concatenate([W1, (W1 @ a_src1)[:, None],
                            (W1 @ a_dst1)[:, None]], axis=1)
    w2aug = np.concatenate([W2, (W2 @ a_src2)[:, None],
                            (W2 @ a_dst2)[:, None]], axis=1)
    iota = np.broadcast_to(np.arange(P, dtype=np.float32), (P, P))
    common = {
        "w1": w1aug.astype(ml_dtypes.bfloat16),
        "w2": w2aug.astype(ml_dtypes.bfloat16),
        "b1r": np.broadcast_to(b1, (P, F)).astype(np.float32).copy(),
        "b2r": np.broadcast_to(b2, (P, F)).astype(np.float32).copy(),
        "wl": Wlin.astype(ml_dtypes.bfloat16),
        "bl": blin.reshape(n_cls, 1).astype(np.float32),
        "iota": iota.astype(ml_dtypes.bfloat16),
        "pcol": np.arange(P, dtype=np.float32).reshape(P, 1),
    }
    in_maps = []
    gid = batch.astype(np.int64)
    for c in range(NC):
        xTc = np.zeros((P, nodes_pad), np.float32)
        xTc[:, :nodes[c]] = x[nbase[c]:nbase[c + 1]].T
        glc = np.full((P, n_tiles), LID_PAD, np.float32)
        nn = np.arange(nodes[c])
        glc[nn % P, nn // P] = gid[nbase[c]:nbase[c + 1]] - gcut[c]
        rc = np.ones((P, 1), np.float32)
        ng = gcut[c + 1] - gcut[c]
        rc[:ng, 0] = 1.0 / np.maximum(gcnt[gcut[c]:gcut[c + 1]], 1.0)
        m = {
            "xT": xTc.astype(ml_dtypes.float8_e4m3),
            "gsl": _wrap16(slo_a[c]), "gdl": _wrap16(dlo_a[c]),
            "gl": glc, "rcnt": rc,
        }
        if TBhi:
            m["gsh"] = _wrap16(shi_a[c])
            m["gdh"] = _wrap16(dhi_a[c])
        m.update(common)
        in_maps.append(m)

    LAST_LAUNCH_WALLS.clear()
    res = _run(ncm, in_maps, list(range(NC)))
    out = np.empty((n_graphs, n_cls), np.float32)
    for c in range(NC):
        lg = res.results[c]["logits"]
        ng = gcut[c + 1] - gcut[c]
        out[gcut[c]:gcut[c + 1]] = lg[:, :ng].T
    return out
